# revision 1
# baseline (speedup 1.0000x reference)
"""Multi-head causal self-attention (B=4, T=2048, C=1024, 16 heads) on 8 trn2 cores.

Sharding: data-parallel over batch (4) x tensor-parallel over heads (2 groups of 8).
Core m handles batch m//2, head group m%2. Host pre-transposes x and the weights so
every on-device matmul consumes operands in natural layout (zero on-device
transposes); the output projection partial sums are pair-reduced on host (+bias).

Per-core pipeline (all matmuls fp32r = FP22 multiply, fp32 PSUM accumulate):
  qT[o,t] = Wq_g @ x^T        (lhsT = Wq_g^T chunks, rhs = x^T chunks)
  kT[o,t] likewise; v[t,o]    (lhsT = x^T chunks, rhs = Wv_g^T)
  scores^T[k,q] per head      (lhsT = kT tile [64,128], rhs = qT tile [64,512])
  p = exp(0.125*scores^T)     (ACT, causal mask via memset + triangle multiply)
  [AV^T | denom] = [v|1]^T @ p (ones column of v gives softmax denominators)
  avT = AV^T * exp(-ln(denom)) broadcast via K=1 ones-matmul
  out_partial = avT^T @ Wp_g^T
"""

import numpy as np

import concourse.bass as bass
import concourse.mybir as mybir
import concourse.tile as tile
from concourse.bass_utils import run_bass_kernel_spmd

F32 = mybir.dt.float32
F32R = mybir.dt.float32r
AF = mybir.ActivationFunctionType
MULT = mybir.AluOpType.mult

B, T, C = 4, 2048, 1024
HEADS, D = 16, 64
GROUPS = 2                  # head groups (tensor parallel)
HPC = HEADS // GROUPS       # heads per core = 8
GC = HPC * D                # group channel width = 512
NKC = T // 128              # Tk chunks = 16
NJ = T // 512               # Tq tiles = 4
CCH = C // 128              # contraction chunks = 8
NSTRIP = T // 512           # phase-1 t strips = 4

_PROGRAM = None


def _patch_drain_chunking():
    """The axon walrus build rejects instructions with >~4 sem waits; Tile's
    kernel-tail drain waits on every live semaphore at once. Split it into a
    chain of drains with <=2 waits each."""
    from bass_rust import VectorClock, ScopedClock

    if getattr(tile.TileContext, "_drain_chunk_patched", False):
        return

    def _drain_and_barrier(self, tick_clock, wait_clock):
        gc_vec = list(tick_clock.global_clock)
        nz = [i for i, t in enumerate(gc_vec) if t > 0]
        CHUNK = 1
        for k in range(0, len(nz), CHUNK):
            keep = set(nz[k:k + CHUNK])
            partial = [gc_vec[i] if i in keep else 0 for i in range(len(gc_vec))]
            d = self.nc.sync.drain()
            wait_clock.add_sem_waits(d.ins, ScopedClock({None: VectorClock(partial)}))
        self.nc.all_engine_barrier()
        assert self.sems is not None
        popped = self.nc._tile_sem_poison_stack.pop()
        assert popped is self._sem_poison
        self.nc.clear_and_free_semaphores(list(self.sems.allocated().values()))
        self.nc.all_engine_barrier()

    tile.TileContext._drain_and_barrier = _drain_and_barrier
    tile.TileContext._drain_chunk_patched = True


def _split_excess_waits(nc, maxw=1, maxw_other=None):
    """Walrus rejects instructions carrying more than ~1 sem wait (proven for
    PE matmul S3_LW and the SP drain at 5). Move excess waits onto same-engine
    NoOps inserted immediately before the instruction (engine streams execute
    in bb order, so semantics are preserved). maxw_other, if set, applies to
    non-PE engines."""
    from bass_rust import InstNoOp

    ctr = 0
    for f in nc.m.functions:
        for bb in f.blocks:
            new_insts = []
            for inst in bb.instructions:
                si = inst.sync_info
                waits = list(si.on_wait) if si and si.on_wait else []
                lim = maxw
                if maxw_other is not None and str(inst.engine) != 'EngineType.PE':
                    lim = maxw_other
                maxw_eff = lim
                if len(waits) > maxw_eff:
                    head, rest = waits[:-maxw_eff], waits[-maxw_eff:]
                    for k in range(0, len(head), maxw_eff):
                        ctr += 1
                        new_insts.append(InstNoOp(
                            name=f"waitnop_{ctr}",
                            engine=inst.engine,
                            sync_info=mybir.SyncInfo(
                                on_wait=head[k:k + maxw_eff], on_update=[]),
                        ))
                    inst.sync_info = mybir.SyncInfo(on_wait=rest, on_update=si.on_update)
                new_insts.append(inst)
            bb.instructions = new_insts
    return ctr


def _build_program():
    _patch_drain_chunking()
    nc = bass.Bass()

    xT_d = nc.declare_dram_parameter("xT", [C, T], F32R, isOutput=False)
    wq_d = nc.declare_dram_parameter("wqT", [C, GC], F32R, isOutput=False)
    wk_d = nc.declare_dram_parameter("wkT", [C, GC], F32R, isOutput=False)
    wv_d = nc.declare_dram_parameter("wvT", [C, GC], F32R, isOutput=False)
    wp_d = nc.declare_dram_parameter("wpT", [GC, C], F32R, isOutput=False)
    out_d = nc.declare_dram_parameter("outp", [T, C], F32, isOutput=True)

    from contextlib import ExitStack

    with tile.TileContext(nc) as tc, ExitStack() as stack:
        cpool = stack.enter_context(tc.tile_pool(name="const", bufs=1))
        qkv_pool = stack.enter_context(tc.tile_pool(name="qkv", bufs=1))

        # additive causal mask: 0 where q >= k, -1e9 where q < k (exp -> 0)
        maskneg = cpool.tile([128, 128], F32)
        nc.gpsimd.memset(maskneg[:, :], 0.0)
        nc.gpsimd.affine_select(
            out=maskneg[:, :], in_=maskneg[:, :],
            compare_op=mybir.AluOpType.is_ge, fill=-1e9, base=0,
            pattern=[[1, 128]], channel_multiplier=-1,
        )
        # DVE cannot encode f32r, so f32r tiles are written by ACT/DMA only
        ones = cpool.tile([128, 128], F32R)
        nc.scalar.activation(ones[64:65, :], ones[64:65, :], AF.Copy, scale=0.0, bias=1.0)

        qT = qkv_pool.tile([128, HPC // 2, T], F32R)   # [c, head-pair, t]
        kT = qkv_pool.tile([128, HPC // 2, T], F32R)
        # v padded with a ones column per head: [t-chunk, head, 65]
        v = qkv_pool.tile([128, NKC, HPC, D + 1], F32R)
        nc.scalar.activation(v[:, :, :, D:D + 1], v[:, :, :, D:D + 1],
                             AF.Copy, scale=0.0, bias=1.0)

        # ---------------- Phase 1: QKV projections ----------------
        with tc.tile_pool(name="w1", bufs=1) as wpool, \
             tc.tile_pool(name="xs", bufs=3) as xpool, \
             tc.tile_pool(name="tmp1", bufs=4) as tmp1, \
             tc.tile_pool(name="ps1", bufs=8, space="PSUM") as ps1:
            wq = wpool.tile([128, CCH, GC], F32R)
            wk = wpool.tile([128, CCH, GC], F32R)
            wv = wpool.tile([128, CCH, GC], F32R)

            for s in range(NSTRIP):
                xs = xpool.tile([128, CCH, 512], F32R)
                nc.sync.dma_start(
                    xs[:, :, :],
                    xT_d[:, 512 * s:512 * (s + 1)].rearrange("(c p) t -> p c t", p=128))
                if s == 0:
                    # batched weight loads, emitted after the first x strip so
                    # the PE can start as soon as wq lands (wq first: q runs first)
                    for w_sb, w_d in ((wq, wq_d), (wk, wk_d), (wv, wv_d)):
                        nc.sync.dma_start(w_sb[:, :, :],
                                          w_d[:, :].rearrange("(c p) o -> p c o", p=128))
                for w_sb, dst in ((wq, qT), (wk, kT)):
                    for o in range(HPC // 2):
                        pq = ps1.tile([128, 512], F32, tag="pp")
                        for c in range(CCH):
                            nc.tensor.matmul(pq[:, :], w_sb[:, c, 128 * o:128 * (o + 1)],
                                             xs[:, c, :], start=(c == 0), stop=(c == CCH - 1))
                        tq = tmp1.tile([128, 512], F32, tag="t1")
                        nc.vector.tensor_copy(tq[:, :], pq[:, :])
                        nc.sync.dma_start(dst[:, o, 512 * s:512 * (s + 1)],
                                          tq[:, :].bitcast(F32R))
                for tt in range(4):
                    pv = ps1.tile([128, 512], F32, tag="pp")
                    for c in range(CCH):
                        nc.tensor.matmul(pv[:, :], xs[:, c, 128 * tt:128 * (tt + 1)],
                                         wv[:, c, :], start=(c == 0), stop=(c == CCH - 1))
                    tv = tmp1.tile([128, 512], F32, tag="t1")
                    nc.vector.tensor_copy(tv[:, :], pv[:, :])
                    nc.sync.dma_start(
                        v[:, 4 * s + tt, :, 0:D],
                        tv[:, :].rearrange("p (h d) -> p h d", h=HPC).bitcast(F32R))

        # ---------------- Phase 2+3: attention + output projection ----------------
        avT = stack.enter_context(tc.tile_pool(name="avt", bufs=1)).tile([128, HPC // 2, T], F32R)
        wp = stack.enter_context(tc.tile_pool(name="wp", bufs=1)).tile([128, GC // 128, C], F32R)
        nc.sync.dma_start(wp[:, :, :], wp_d[:, :].rearrange("(c p) o -> p c o", p=128))

        with tc.tile_pool(name="pt", bufs=8) as pt_pool, \
             tc.tile_pool(name="dd", bufs=4) as d_pool, \
             tc.tile_pool(name="rr", bufs=3) as r_pool, \
             tc.tile_pool(name="avtmp", bufs=3) as avtmp_pool, \
             tc.tile_pool(name="ob", bufs=4) as out_pool, \
             tc.tile_pool(name="ps_s", bufs=4, space="PSUM") as ps_s, \
             tc.tile_pool(name="ps_av", bufs=2, space="PSUM") as ps_av, \
             tc.tile_pool(name="ps_bc", bufs=1, space="PSUM") as ps_bc, \
             tc.tile_pool(name="ps_o", bufs=1, space="PSUM") as ps_o:

            for j in range(NJ):
                for hp in range(HPC // 2):
                    nkc = 4 * (j + 1)
                    # both heads of the pair run interleaved: their scores
                    # matmuls sit in adjacent PE slots with disjoint row
                    # groups (K=64 at partition 0 vs 64) and overlap on HW
                    av0 = ps_av.tile([65, 512], F32, tag="av")
                    av1 = ps_av.tile([65, 512], F32, tag="av")
                    avs = [av0, av1]
                    def emit_scores_exp(i):
                        out = []
                        for par in range(2):
                            pb = 64 * par
                            sps = ps_s.tile([128, 512], F32, tag="s")
                            nc.tensor.matmul(
                                sps[:, :],
                                kT[pb:pb + 64, hp, 128 * i:128 * (i + 1)],
                                qT[pb:pb + 64, hp, 512 * j:512 * (j + 1)],
                                start=True, stop=True)
                            ptile = pt_pool.tile([128, 512], F32R, tag="pt")
                            roff = 128 * i - 512 * j
                            if roff >= 0:
                                # diagonal tile: add -1e9 above the diagonal in
                                # PSUM, then exp only the columns [roff:512] the
                                # AV matmul will consume (cols [0:roff] are
                                # fully masked and skipped outright)
                                nc.vector.tensor_tensor(
                                    sps[:, roff:roff + 128], sps[:, roff:roff + 128],
                                    maskneg[:, :], op=mybir.AluOpType.add)
                                nc.scalar.activation(ptile[:, roff:512], sps[:, roff:512],
                                                     AF.Exp, scale=0.125)
                            else:
                                roff = 0
                                nc.scalar.activation(ptile[:, :], sps[:, :], AF.Exp, scale=0.125)
                            out.append((ptile, roff))
                        return out

                    def emit_av(i, pts):
                        for par in range(2):
                            ptile, roff = pts[par]
                            nc.tensor.matmul(avs[par][:, roff:512], v[:, i, 2 * hp + par, :],
                                             ptile[:, roff:512],
                                             start=(i == 0), stop=(i == nkc - 1))

                    # one-chunk software pipeline: chunk i+1's scores sit ahead
                    # of chunk i's AV matmuls in the PE stream, so AV never
                    # waits out the exp latency
                    prev = emit_scores_exp(0)
                    for i in range(1, nkc):
                        cur = emit_scores_exp(i)
                        emit_av(i - 1, prev)
                        prev = cur
                    emit_av(nkc - 1, prev)
                    for par in range(2):
                        av = avs[par]
                        # single DVE copy frees the AV PSUM bank immediately so
                        # the next head pair's AV matmuls are not gated on the
                        # whole normalize chain
                        avr = avtmp_pool.tile([65, 512], F32, tag="avr")
                        nc.vector.tensor_copy(avr[:, :], av[:, :])
                        # softmax denominators: r = exp(-ln(denom)), broadcast via K=1 matmul
                        dt_ = d_pool.tile([65, 512], F32R, tag="d")
                        nc.scalar.activation(dt_[64:65, :], avr[64:65, :], AF.Ln)
                        nc.scalar.activation(dt_[64:65, :], dt_[64:65, :], AF.Exp, scale=-1.0)
                        bc = ps_bc.tile([128, 512], F32, tag="bc")
                        nc.tensor.matmul(bc[:, :], ones[64:65, :], dt_[64:65, :],
                                         start=True, stop=True)
                        rb = r_pool.tile([64, 512], F32, tag="r")
                        nc.vector.tensor_copy(rb[:, :], bc[0:64, :])
                        avf = avtmp_pool.tile([64, 512], F32, tag="avf")
                        nc.vector.tensor_tensor(avf[:, :], avr[0:64, :], rb[:, :], op=MULT)
                        # DMA moves lanes 0:64 to the destination partitions
                        nc.sync.dma_start(avT[64 * par:64 * par + 64, hp, 512 * j:512 * (j + 1)],
                                          avf[:, :].bitcast(F32R))

                # output projection for the t-tiles whose avT columns just completed
                for tt in range(4 * j, 4 * (j + 1)):
                    ob = out_pool.tile([128, C], F32, tag="ob")
                    for o2 in range(2):
                        po = ps_o.tile([128, 512], F32, tag="o")
                        for c4 in range(GC // 128):
                            nc.tensor.matmul(po[:, :], avT[:, c4, 128 * tt:128 * (tt + 1)],
                                             wp[:, c4, 512 * o2:512 * (o2 + 1)],
                                             start=(c4 == 0), stop=(c4 == GC // 128 - 1))
                        nc.vector.tensor_copy(ob[:, 512 * o2:512 * (o2 + 1)], po[:, :])
                    nc.sync.dma_start(out_d[128 * tt:128 * (tt + 1), :], ob[:, :])
    _split_excess_waits(nc)
    return nc


def _get_program():
    global _PROGRAM
    if _PROGRAM is None:
        _PROGRAM = _build_program()
    return _PROGRAM


def _make_in_maps(x, Wk, Wq, Wv, Wp):
    x = np.asarray(x, dtype=np.float32)
    Wk = np.asarray(Wk, dtype=np.float32)
    Wq = np.asarray(Wq, dtype=np.float32)
    Wv = np.asarray(Wv, dtype=np.float32)
    Wp = np.asarray(Wp, dtype=np.float32)
    in_maps = []
    for core in range(8):
        b, g = core // GROUPS, core % GROUPS
        rows = slice(GC * g, GC * (g + 1))
        in_maps.append({
            "xT": np.ascontiguousarray(x[b].T),                 # [C, T]
            "wqT": np.ascontiguousarray(Wq[rows, :].T),         # [C, GC]
            "wkT": np.ascontiguousarray(Wk[rows, :].T),
            "wvT": np.ascontiguousarray(Wv[rows, :].T),
            "wpT": np.ascontiguousarray(Wp[:, rows].T),         # [GC, C]
        })
    return in_maps


def run(x, Wk, Wq, Wv, Wp, bp, trace=False, **spmd_kwargs):
    nc = _get_program()
    in_maps = _make_in_maps(x, Wk, Wq, Wv, Wp)
    res = run_bass_kernel_spmd(nc, in_maps, list(range(8)), trace=trace, **spmd_kwargs)
    bp = np.asarray(bp, dtype=np.float32)
    out = np.empty((B, T, C), dtype=np.float32)
    for b in range(B):
        out[b] = res.results[GROUPS * b]["outp"] + res.results[GROUPS * b + 1]["outp"] + bp
    return out, res


def kernel(x, Wk, Wq, Wv, Wp, bp):
    out, _ = run(x, Wk, Wq, Wv, Wp, bp)
    return out



# revision 39
# speedup vs baseline: 1.6018x; 1.6018x over previous
"""Multi-head causal self-attention (B=4, T=2048, C=1024, 16 heads) on 8 trn2 cores.

Sharding: data-parallel over batch (4) x tensor-parallel over heads (2 groups of 8).
Core m handles batch m//2, head group m%2. Host pre-transposes x and the weights;
the output projection partial sums are pair-reduced on host (+bias).

Per-core pipeline (all matmul operands bf16, fp32 PSUM accumulate):
  qT[d2,t], kT[d2,t] = W @ x^T per head-pair; v[t,h,d] = x @ Wv^T  (+ ones col)
  per (q-chunk, head-pair): scores^T block [k,q] = kT-chunk^T-slice @ qT-slice
  p = exp(0.125*scores) (one ACT op per 2-head 4-chunk group), tril mask by
  bf16 multiply on gpsimd, then transposed AV: av[q, d+1] += p_blk.T @ v~ with
  the ones column producing softmax denominators in column d.
  normalize = DVE reciprocal + per-partition tensor_scalar mult -> av_sb [q, d2]
  PE-transpose -> avT [d2, t]; out[t,:] = avT-chunks^T @ Wp^T, DMA to DRAM.
QKV strips and the output projection are interleaved into the attention loop so
the PE stream stays dense while ACT exp runs.
"""

import numpy as np

import concourse.bass as bass
import concourse.mybir as mybir
import concourse.tile as tile
from concourse.bass_utils import run_bass_kernel_spmd

F32 = mybir.dt.float32
BF16 = mybir.dt.bfloat16
F8 = mybir.dt.float8e4
AF = mybir.ActivationFunctionType
MULT = mybir.AluOpType.mult
DR = mybir.MatmulPerfMode.DoubleRow

# QKV projections in fp8e4m3 DoubleRow (2x PE throughput). Host pre-scales the
# QKV weights by WSCALE to dodge fp8 subnormals; compensated via the exp scale
# (q,k) and the normalize epilogue (v).
USE_FP8_QKV = False
WSCALE = 32.0

B, T, C = 4, 2048, 1024
HEADS, D = 16, 64
GROUPS = 2                  # head groups (tensor parallel)
HPC = HEADS // GROUPS       # heads per core = 8
NPAIR = HPC // 2            # head pairs per core = 4
GC = HPC * D                # group channel width = 512
NQC = T // 128              # q chunks (and k chunks) = 16
CCH = C // 128              # contraction chunks = 8
NSTRIP = T // 512           # qkv t strips = 4

_PROGRAM = None


def _patch_drain_chunking():
    """The axon walrus build rejects instructions with >~4 sem waits; Tile's
    kernel-tail drain waits on every live semaphore at once. Split it into a
    chain of drains with <=2 waits each."""
    from bass_rust import VectorClock, ScopedClock

    if getattr(tile.TileContext, "_drain_chunk_patched", False):
        return

    def _drain_and_barrier(self, tick_clock, wait_clock):
        gc_vec = list(tick_clock.global_clock)
        nz = [i for i, t in enumerate(gc_vec) if t > 0]
        CHUNK = 1
        for k in range(0, len(nz), CHUNK):
            keep = set(nz[k:k + CHUNK])
            partial = [gc_vec[i] if i in keep else 0 for i in range(len(gc_vec))]
            d = self.nc.sync.drain()
            wait_clock.add_sem_waits(d.ins, ScopedClock({None: VectorClock(partial)}))
        self.nc.all_engine_barrier()
        assert self.sems is not None
        popped = self.nc._tile_sem_poison_stack.pop()
        assert popped is self._sem_poison
        self.nc.clear_and_free_semaphores(list(self.sems.allocated().values()))
        self.nc.all_engine_barrier()

    tile.TileContext._drain_and_barrier = _drain_and_barrier
    tile.TileContext._drain_chunk_patched = True


def _split_excess_waits(nc, maxw=1, maxw_other=None):
    """Walrus rejects instructions carrying more than ~1 sem wait. Move excess
    waits onto same-engine NoOps inserted immediately before the instruction
    (engine streams execute in bb order, so semantics are preserved)."""
    from bass_rust import InstNoOp

    ctr = 0
    for f in nc.m.functions:
        for bb in f.blocks:
            new_insts = []
            for inst in bb.instructions:
                si = inst.sync_info
                waits = list(si.on_wait) if si and si.on_wait else []
                lim = maxw
                if maxw_other is not None and str(inst.engine) != 'EngineType.PE':
                    lim = maxw_other
                maxw_eff = lim
                if len(waits) > maxw_eff:
                    head, rest = waits[:-maxw_eff], waits[-maxw_eff:]
                    for k in range(0, len(head), maxw_eff):
                        ctr += 1
                        new_insts.append(InstNoOp(
                            name=f"waitnop_{ctr}",
                            engine=inst.engine,
                            sync_info=mybir.SyncInfo(
                                on_wait=head[k:k + maxw_eff], on_update=[]),
                        ))
                    inst.sync_info = mybir.SyncInfo(on_wait=rest, on_update=si.on_update)
                new_insts.append(inst)
            bb.instructions = new_insts
    return ctr


def _build_program(split_waits=True):
    _patch_drain_chunking()
    nc = bass.Bass()

    XDT = F8 if USE_FP8_QKV else BF16
    xT_d = nc.declare_dram_parameter("xT", [C, T], XDT, isOutput=False)
    wq_d = nc.declare_dram_parameter("wqT", [C, GC], XDT, isOutput=False)
    wk_d = nc.declare_dram_parameter("wkT", [C, GC], XDT, isOutput=False)
    wv_d = nc.declare_dram_parameter("wvT", [C, GC], XDT, isOutput=False)
    wp_d = nc.declare_dram_parameter("wpT", [GC, C], BF16, isOutput=False)
    out_d = nc.declare_dram_parameter("outp", [T, C], F32, isOutput=True)
    exp_scale = 0.125 / (WSCALE * WSCALE) if USE_FP8_QKV else 0.125
    v_unscale = 1.0 / WSCALE if USE_FP8_QKV else None

    from contextlib import ExitStack

    with tile.TileContext(nc) as tc, ExitStack() as stack:
        ep = stack.enter_context

        cpool = ep(tc.tile_pool(name="const", bufs=1))
        # tril[k, q] = 1 where q >= k else 0: multiplied into the diagonal
        # p blocks post-exp on the otherwise-idle gpsimd engine
        tril = cpool.tile([128, 128], BF16)
        nc.gpsimd.memset(tril[:, :], 1.0)
        nc.gpsimd.affine_select(
            out=tril[:, :], in_=tril[:, :],
            compare_op=mybir.AluOpType.is_ge, fill=0.0, base=0,
            pattern=[[1, 128]], channel_multiplier=-1,
        )
        # identity for PE transpose
        ident = cpool.tile([128, 128], BF16)
        nc.gpsimd.memset(ident[:, :], 1.0)
        nc.gpsimd.affine_select(
            out=ident[:, :], in_=ident[:, :],
            compare_op=mybir.AluOpType.is_ge, fill=0.0, base=0,
            pattern=[[1, 128]], channel_multiplier=-1,
        )
        nc.gpsimd.affine_select(
            out=ident[:, :], in_=ident[:, :],
            compare_op=mybir.AluOpType.is_ge, fill=0.0, base=0,
            pattern=[[-1, 128]], channel_multiplier=1,
        )

        # persistent activation buffers
        big = ep(tc.tile_pool(name="big", bufs=1))
        qT = big.tile([128, NPAIR, T], BF16)    # [d-of-pair, pair, t]
        kT = big.tile([128, NPAIR, T], BF16)
        v = big.tile([128, NQC, HPC, D + 1], BF16)  # [t%128, t-chunk, head, d|1]
        avT = big.tile([128, NPAIR, T], BF16)   # [d-of-pair(gc-chunk), pair, t]
        nc.vector.memset(v[:, :, :, D:D + 1], 1.0)
        # warm the ACT exp table before the critical path
        warm = cpool.tile([128, 1], F32)
        nc.vector.memset(warm[:, :], 0.0)
        nc.scalar.activation(warm[:, :], warm[:, :], AF.Exp, scale=0.0)

        # weights
        wpool = ep(tc.tile_pool(name="w", bufs=1))
        wq = wpool.tile([128, CCH, GC], XDT)
        wk = wpool.tile([128, CCH, GC], XDT)
        wv = wpool.tile([128, CCH, GC], XDT)
        wp = wpool.tile([128, GC // 128, C], BF16)

        xpool = ep(tc.tile_pool(name="xs", bufs=1))
        ppool = ep(tc.tile_pool(name="pt", bufs=4))
        avsb_pool = ep(tc.tile_pool(name="avsb", bufs=3))
        ob_pool = ep(tc.tile_pool(name="ob", bufs=2))
        rc_pool = ep(tc.tile_pool(name="rc", bufs=3))
        ps_s = ep(tc.tile_pool(name="ps_s", bufs=2, space="PSUM"))
        ps_av_pool = ep(tc.tile_pool(name="ps_av", bufs=1, space="PSUM"))
        ps_m = ep(tc.tile_pool(name="ps_m", bufs=2, space="PSUM"))

        # av accumulators: one bank per head-parity (concurrent accumulation
        # groups must not share a PSUM bank: start=True zeroes the whole bank),
        # with two pair-slots per bank used strictly sequentially.
        av_a = ps_av_pool.tile([128, 2, D + 1], F32)   # h2 = 0, [slot, d|denom]
        av_b = ps_av_pool.tile([128, 2, D + 1], F32)   # h2 = 1
        av_banks = (av_a, av_b)

        xs = xpool.tile([128, NSTRIP, CCH, 512], XDT)

        def emit_qkv_mm(pq, s, kind, o, j):
            """Micro-step j (0..7) of one 128-row projection group."""
            w_sb = {"q": wq, "k": wk, "v": wv}[kind]
            if USE_FP8_QKV:
                half, g = j // 4, j % 4
                if kind == "v":
                    lhsT = xs[:, s, 2 * g:2 * g + 2,
                              128 * o + 64 * half:128 * o + 64 * half + 64]
                    rhs = wv[:, 2 * g:2 * g + 2, :]
                else:
                    lhsT = w_sb[:, 2 * g:2 * g + 2,
                                128 * o + 64 * half:128 * o + 64 * half + 64]
                    rhs = xs[:, s, 2 * g:2 * g + 2, :]
                nc.tensor.matmul(pq[64 * half:64 * half + 64, :], lhsT, rhs,
                                 start=(g == 0), stop=(g == 3),
                                 perf_mode=DR, skip_group_check=True)
            else:
                c = j
                if kind == "v":
                    nc.tensor.matmul(pq[:, :], xs[:, s, c, 128 * o:128 * (o + 1)],
                                     wv[:, c, :], start=(c == 0),
                                     stop=(c == CCH - 1), skip_group_check=True)
                else:
                    nc.tensor.matmul(pq[:, :], w_sb[:, c, 128 * o:128 * (o + 1)],
                                     xs[:, s, c, :], start=(c == 0),
                                     stop=(c == CCH - 1), skip_group_check=True)

        def emit_qkv_fin(pq, s, kind, o):
            if kind == "v":
                nc.vector.tensor_copy(
                    v[:, 4 * s + o, :, 0:D],
                    pq[:, :].rearrange("p (h d) -> p h d", h=HPC))
            else:
                dst = qT if kind == "q" else kT
                nc.vector.tensor_copy(dst[:, o, 512 * s:512 * (s + 1)], pq[:, :])

        def emit_qkv_group(s, kind, o):
            pq = ps_m.tile([128, 512], F32, tag="m")
            for j in range(8):
                emit_qkv_mm(pq, s, kind, o, j)
            emit_qkv_fin(pq, s, kind, o)

        # ---- startup: fine-grained first DMAs, V first so the PE can start
        # after one 128-col x quarter + half of Wv instead of the whole strip.
        def dma_x_piece(s, lo, hi):
            nc.sync.dma_start(
                xs[:, s, :, lo:hi],
                xT_d[:, 512 * s + lo:512 * s + hi].rearrange(
                    "(c p) t -> p c t", p=128))

        wv_r = wv_d[:, :].rearrange("(c p) o -> p c o", p=128)
        wq_r = wq_d[:, :].rearrange("(c p) o -> p c o", p=128)
        nc.sync.dma_start(wv[:, 0:2, :], wv_r[:, 0:2, :])
        dma_x_piece(0, 0, 128)
        nc.sync.dma_start(wv[:, 2:5, :], wv_r[:, 2:5, :])
        nc.sync.dma_start(wv[:, 5:8, :], wv_r[:, 5:8, :])
        dma_x_piece(0, 128, 256)
        nc.sync.dma_start(wq[:, 0:2, :], wq_r[:, 0:2, :])
        dma_x_piece(0, 256, 384)
        nc.sync.dma_start(wq[:, 2:4, :], wq_r[:, 2:4, :])
        dma_x_piece(0, 384, 512)
        nc.sync.dma_start(wq[:, 4:6, :], wq_r[:, 4:6, :])
        nc.sync.dma_start(wq[:, 6:8, :], wq_r[:, 6:8, :])
        nc.sync.dma_start(wk[:, :, :], wk_d[:, :].rearrange("(c p) o -> p c o", p=128))
        for o in range(4):
            emit_qkv_group(0, "v", o)
        for s in range(1, NSTRIP):
            dma_x_piece(s, 0, 512)
        nc.sync.dma_start(wp[:, :, :], wp_d[:, :].rearrange("(g p) o -> p g o", p=128))
        for kind in ("q", "k"):
            for o in range(4):
                emit_qkv_group(0, kind, o)

        # ---------------- attention + projection pipeline ----------------
        def emit_scores_exp(qc, pair, g, w, split_exp=False):
            sps = ps_s.tile([128, 2, 4, 128], F32, tag="s")
            for h2 in range(2):
                pb = 64 * h2
                for i in range(w):
                    kc = 4 * g + i
                    nc.tensor.matmul(
                        sps[:, h2, i, :],
                        kT[pb:pb + 64, pair, 128 * kc:128 * (kc + 1)],
                        qT[pb:pb + 64, pair, 128 * qc:128 * (qc + 1)],
                        start=True, stop=True)
            p = ppool.tile([128, 2, 4, 128], BF16, tag="p")
            if split_exp:
                # per-parity exp halves shorten the dependence chain at the
                # kernel tail (av h2=0 starts while h2=1 still exponentiates)
                for h2 in range(2):
                    nc.scalar.activation(p[:, h2, 0:w, :], sps[:, h2, 0:w, :],
                                         AF.Exp, scale=exp_scale)
            else:
                nc.scalar.activation(p[:, :, 0:w, :], sps[:, :, 0:w, :],
                                     AF.Exp, scale=exp_scale)
            # tril-mask the diagonal block (kc == qc) post-exp, on gpsimd
            if 4 * g <= qc < 4 * (g + 1):
                i = qc - 4 * g
                for h2 in range(2):
                    nc.gpsimd.tensor_tensor(p[:, h2, i, :], p[:, h2, i, :],
                                            tril[:, :], op=MULT)
            return p

        def emit_av(qc, pair, g, w, p):
            slot = pair % 2
            nkc = qc + 1
            for h2 in range(2):
                head = 2 * pair + h2
                for i in range(w):
                    kc = 4 * g + i
                    nc.tensor.matmul(
                        av_banks[h2][:, slot, :], p[:, h2, i, :],
                        v[:, kc, head, :],
                        start=(kc == 0), stop=(kc == nkc - 1),
                        skip_group_check=True)

        def emit_normalize(qc, pair):
            slot = pair % 2
            av_sb = avsb_pool.tile([128, 128], BF16, tag="avsb")
            rc = rc_pool.tile([128, 2], F32, tag="rc")
            for h2 in range(2):
                nc.vector.reciprocal(rc[:, h2:h2 + 1], av_banks[h2][:, slot, D:D + 1])
                if v_unscale is None:
                    nc.vector.tensor_scalar(
                        out=av_sb[:, 64 * h2:64 * (h2 + 1)],
                        in0=av_banks[h2][:, slot, 0:D],
                        scalar1=rc[:, h2:h2 + 1], scalar2=None, op0=MULT)
                else:
                    nc.vector.tensor_scalar(
                        out=av_sb[:, 64 * h2:64 * (h2 + 1)],
                        in0=av_banks[h2][:, slot, 0:D],
                        scalar1=rc[:, h2:h2 + 1], scalar2=v_unscale,
                        op0=MULT, op1=MULT)
            return av_sb

        def emit_transpose(qc, pair, av_sb):
            if qc < NQC - 2:
                # xbar DMA transpose: frees PE + DVE; ~2.5us issue latency is
                # hidden by the 2-pair transpose lag
                nc.sync.dma_start_transpose(
                    avT[:, pair, 128 * qc:128 * (qc + 1)], av_sb[:, :])
            else:
                # tail q-chunks stay on the PE to keep the flush chain short
                tp = ps_m.tile([128, 128], BF16, tag="m")
                nc.tensor.transpose(tp[:, :], av_sb[:, :], ident[:, :])
                nc.vector.tensor_copy(avT[:, pair, 128 * qc:128 * (qc + 1)], tp[:, :])

        def emit_proj(tt, split_dma=False):
            ob = ob_pool.tile([128, C], F32, tag="ob")
            for o2 in range(2):
                po = ps_m.tile([128, 512], F32, tag="m")
                for c4 in range(GC // 128):
                    nc.tensor.matmul(po[:, :], avT[:, c4, 128 * tt:128 * (tt + 1)],
                                     wp[:, c4, 512 * o2:512 * (o2 + 1)],
                                     start=(c4 == 0), stop=(c4 == GC // 128 - 1))
                nc.vector.tensor_copy(ob[:, 512 * o2:512 * (o2 + 1)], po[:, :])
                if split_dma:
                    nc.sync.dma_start(
                        out_d[128 * tt:128 * (tt + 1), 512 * o2:512 * (o2 + 1)],
                        ob[:, 512 * o2:512 * (o2 + 1)])
            if not split_dma:
                nc.sync.dma_start(out_d[128 * tt:128 * (tt + 1), :], ob[:, :])

        # item stream with a 1-item software pipeline (av lags scores by one)
        items = []
        first_item_of_qc = {}
        for qc in range(NQC):
            nkc = qc + 1
            ngroups = (nkc + 3) // 4
            first_item_of_qc[qc] = len(items)
            for pair in range(NPAIR):
                for g in range(ngroups):
                    w = min(4, nkc - 4 * g)
                    items.append((qc, pair, g, w))
        first_item_of_qc[NQC] = len(items)

        # strip fillers at single-matmul granularity: the per-item PE deficit
        # vs ACT exp is ~450 ns, so whole 1.7 us QKV groups cause run-ahead
        # jitter against the 2-deep sps rotation. A credit model doles out
        # individual accumulation matmuls (213 ns each) to keep the PE stream
        # dense and smooth; strip s must complete before qc = 4s.
        MM = 107.0 if USE_FP8_QKV else 213.0
        strip_micro = []     # (strip, fn, pe_cost, kind: 'mm0'|'mm'|'fin')

        def make_strip_micro():
            for s in range(1, NSTRIP):
                for kind in ("q", "k", "v"):
                    for o in range(4):
                        pq_cell = []
                        for j in range(8):
                            def mm(s=s, kind=kind, o=o, j=j, pq_cell=pq_cell):
                                if j == 0:
                                    pq_cell.append(
                                        ps_m.tile([128, 512], F32, tag="m", name="pq"))
                                emit_qkv_mm(pq_cell[-1], s, kind, o, j)
                            strip_micro.append(
                                (s, mm, MM, "mm0" if j == 0 else "mm"))

                        def fin(s=s, kind=kind, o=o, pq_cell=pq_cell):
                            emit_qkv_fin(pq_cell[-1], s, kind, o)
                        strip_micro.append((s, fin, 0.0, "fin"))

        make_strip_micro()
        strip_ptr = [0]
        credit = [0.0]

        def _emit_next():
            s, fn, cost, k = strip_micro[strip_ptr[0]]
            fn()
            credit[0] -= cost
            strip_ptr[0] += 1

        def pace():
            while strip_ptr[0] < len(strip_micro):
                s, fn, cost, k = strip_micro[strip_ptr[0]]
                if credit[0] < cost and cost > 0:
                    break
                _emit_next()

        def close_open_group():
            # a mid-accumulation strip group holds a ps_m slot; any other
            # ps_m allocation while it is open can deadlock the in-order PE
            # queue on slot reuse, so finish the group first
            while strip_ptr[0] < len(strip_micro) and \
                    strip_micro[strip_ptr[0]][3] != "mm0":
                _emit_next()

        def force_strip(s_done):
            # everything belonging to strips <= s_done must be emitted now
            while strip_ptr[0] < len(strip_micro) and \
                    strip_micro[strip_ptr[0]][0] <= s_done:
                _emit_next()

        prev = None           # (qc, pair, g, w, p)
        pending_norm = {}     # (qc, pair) -> av_sb awaiting transpose

        def drain_transposes(upto_idx):
            # transpose every pending pair whose order index is <= upto_idx
            for key in sorted(pending_norm):
                if key[0] * NPAIR + key[1] <= upto_idx:
                    emit_transpose(key[0], key[1], pending_norm.pop(key))
                    credit[0] -= 53.0

        last_pair_key = None
        for idx, (qc, pair, g, w) in enumerate(items):
            if qc % 4 == 0 and pair == 0 and g == 0 and qc // 4 >= 1:
                force_strip(qc // 4)
                credit[0] = max(credit[0], -2000.0)
            if (qc, pair) != last_pair_key:
                if qc >= 12:
                    # proj + transpose both allocate ps_m at these hooks; an
                    # open strip group there could deadlock the slot rotation
                    close_open_group()
                drain_transposes(qc * NPAIR + pair - 2)
                # output projection deferred to late q-chunks where attention
                # leaves the PE under-filled (exp dominates per-item there)
                if qc >= 12:
                    tt = 4 * (qc - 12) + pair
                    if qc < 15 or pair in (0, 1):
                        emit_proj(tt)          # tt 0..13
                        credit[0] -= 2 * 853.0
                    elif pair == 3:
                        emit_proj(14)
                        credit[0] -= 2 * 853.0
                last_pair_key = (qc, pair)
            # per-item ACT-vs-PE deficit feeds the filler credit
            credit[0] += (213.3 * w + 245.0) - (160.8 * w)
            pace()

            p = emit_scores_exp(qc, pair, g, w,
                                split_exp=(idx == len(items) - 1))
            if prev is not None:
                pqc, ppair, pg, pw, pp_ = prev
                emit_av(pqc, ppair, pg, pw, pp_)
                if 4 * (pg + 1) >= pqc + 1:  # last group of that pair
                    pending_norm[(pqc, ppair)] = emit_normalize(pqc, ppair)
            prev = (qc, pair, g, w, p)

        # flush: interleave proj(15)'s pair-3-independent matmuls with the
        # final transpose chain (alloc order keeps the 2-slot ps_m rotation
        # deadlock-free: tp(15,2), po0, tp(15,3), po0-close, po1)
        pqc, ppair, pg, pw, pp_ = prev
        emit_av(pqc, ppair, pg, pw, pp_)
        pending_norm[(pqc, ppair)] = emit_normalize(pqc, ppair)
        key = (NQC - 1, 2)
        if key in pending_norm:
            emit_transpose(key[0], key[1], pending_norm.pop(key))
        tt = NQC - 1
        ob = ob_pool.tile([128, C], F32, tag="ob")
        po0 = ps_m.tile([128, 512], F32, tag="m")
        for c4 in range(3):
            nc.tensor.matmul(po0[:, :], avT[:, c4, 128 * tt:128 * (tt + 1)],
                             wp[:, c4, 0:512], start=(c4 == 0), stop=False,
                             skip_group_check=True)
        drain_transposes(NQC * NPAIR)
        nc.tensor.matmul(po0[:, :], avT[:, 3, 128 * tt:128 * (tt + 1)],
                         wp[:, 3, 0:512], start=False, stop=True,
                         skip_group_check=True)
        nc.vector.tensor_copy(ob[:, 0:512], po0[:, :])
        nc.sync.dma_start(out_d[128 * tt:128 * (tt + 1), 0:512], ob[:, 0:512])
        po1 = ps_m.tile([128, 512], F32, tag="m")
        for c4 in range(GC // 128):
            nc.tensor.matmul(po1[:, :], avT[:, c4, 128 * tt:128 * (tt + 1)],
                             wp[:, c4, 512:1024], start=(c4 == 0),
                             stop=(c4 == GC // 128 - 1), skip_group_check=True)
        nc.vector.tensor_copy(ob[:, 512:1024], po1[:, :])
        nc.sync.dma_start(out_d[128 * tt:128 * (tt + 1), 512:1024], ob[:, 512:1024])

    if split_waits:
        _split_excess_waits(nc)
    return nc


def _get_program():
    global _PROGRAM
    if _PROGRAM is None:
        _PROGRAM = _build_program()
    return _PROGRAM


def _make_in_maps(x, Wk, Wq, Wv, Wp):
    import ml_dtypes
    bf = ml_dtypes.bfloat16
    xdt = ml_dtypes.float8_e4m3 if USE_FP8_QKV else bf
    ws = WSCALE if USE_FP8_QKV else 1.0
    x = np.asarray(x, dtype=np.float32)
    Wk = np.asarray(Wk, dtype=np.float32)
    Wq = np.asarray(Wq, dtype=np.float32)
    Wv = np.asarray(Wv, dtype=np.float32)
    Wp = np.asarray(Wp, dtype=np.float32)
    in_maps = []
    for core in range(8):
        b, g = core // GROUPS, core % GROUPS
        rows = slice(GC * g, GC * (g + 1))
        in_maps.append({
            "xT": np.ascontiguousarray(x[b].T).astype(xdt),           # [C, T]
            "wqT": np.ascontiguousarray(Wq[rows, :].T * ws).astype(xdt),
            "wkT": np.ascontiguousarray(Wk[rows, :].T * ws).astype(xdt),
            "wvT": np.ascontiguousarray(Wv[rows, :].T * ws).astype(xdt),
            "wpT": np.ascontiguousarray(Wp[:, rows].T).astype(bf),    # [GC, C]
        })
    return in_maps


def run(x, Wk, Wq, Wv, Wp, bp, trace=False, **spmd_kwargs):
    nc = _get_program()
    in_maps = _make_in_maps(x, Wk, Wq, Wv, Wp)
    res = run_bass_kernel_spmd(nc, in_maps, list(range(8)), trace=trace, **spmd_kwargs)
    bp = np.asarray(bp, dtype=np.float32)
    out = np.empty((B, T, C), dtype=np.float32)
    for b in range(B):
        out[b] = res.results[GROUPS * b]["outp"] + res.results[GROUPS * b + 1]["outp"] + bp
    return out, res


def kernel(x, Wk, Wq, Wv, Wp, bp):
    out, _ = run(x, Wk, Wq, Wv, Wp, bp)
    return out


# revision 64
# speedup vs baseline: 1.6356x; 1.0211x over previous
"""Multi-head causal self-attention (B=4, T=2048, C=1024, 16 heads) on 8 trn2 cores.

Sharding: data-parallel over batch (4) x tensor-parallel over heads (2 groups of 8).
Core m handles batch m//2, head group m%2. Host pre-transposes x and the weights;
the output projection partial sums are pair-reduced on host (+bias).

Per-core pipeline (all matmul operands bf16, fp32 PSUM accumulate):
  qT[d2,t], kT[d2,t] = W @ x^T per head-pair; v[t,h,d] = x @ Wv^T  (+ ones col)
  per (q-chunk, head-pair): scores^T block [k,q] = kT-chunk^T-slice @ qT-slice
  p = exp(0.125*scores) (one ACT op per 2-head 4-chunk group), tril mask by
  bf16 multiply on gpsimd, then transposed AV: av[q, d+1] += p_blk.T @ v~ with
  the ones column producing softmax denominators in column d.
  normalize = DVE reciprocal + per-partition tensor_scalar mult -> av_sb [q, d2]
  PE-transpose -> avT [d2, t]; out[t,:] = avT-chunks^T @ Wp^T, DMA to DRAM.
QKV strips and the output projection are interleaved into the attention loop so
the PE stream stays dense while ACT exp runs.
"""

import numpy as np

import concourse.bass as bass
import concourse.mybir as mybir
import concourse.tile as tile
from concourse.bass_utils import run_bass_kernel_spmd

F32 = mybir.dt.float32
BF16 = mybir.dt.bfloat16
F8 = mybir.dt.float8e4
AF = mybir.ActivationFunctionType
MULT = mybir.AluOpType.mult
DR = mybir.MatmulPerfMode.DoubleRow

# QKV projections in fp8e4m3 DoubleRow (2x PE throughput). Host pre-scales the
# QKV weights by WSCALE to dodge fp8 subnormals; compensated via the exp scale
# (q,k) and the normalize epilogue (v).
USE_FP8_QKV = False
WSCALE = 32.0

B, T, C = 4, 2048, 1024
HEADS, D = 16, 64
GROUPS = 2                  # head groups (tensor parallel)
HPC = HEADS // GROUPS       # heads per core = 8
NPAIR = HPC // 2            # head pairs per core = 4
GC = HPC * D                # group channel width = 512
NQC = T // 128              # q chunks (and k chunks) = 16
CCH = C // 128              # contraction chunks = 8
NSTRIP = T // 512           # qkv t strips = 4

_PROGRAM = None


def _patch_drain_chunking():
    """The axon walrus build rejects instructions with >~4 sem waits; Tile's
    kernel-tail drain waits on every live semaphore at once. Split it into a
    chain of drains with <=2 waits each."""
    from bass_rust import VectorClock, ScopedClock

    if getattr(tile.TileContext, "_drain_chunk_patched", False):
        return

    def _drain_and_barrier(self, tick_clock, wait_clock):
        gc_vec = list(tick_clock.global_clock)
        nz = [i for i, t in enumerate(gc_vec) if t > 0]
        CHUNK = 2
        for k in range(0, len(nz), CHUNK):
            keep = set(nz[k:k + CHUNK])
            partial = [gc_vec[i] if i in keep else 0 for i in range(len(gc_vec))]
            d = self.nc.sync.drain()
            wait_clock.add_sem_waits(d.ins, ScopedClock({None: VectorClock(partial)}))
        self.nc.all_engine_barrier()
        assert self.sems is not None
        popped = self.nc._tile_sem_poison_stack.pop()
        assert popped is self._sem_poison
        self.nc.clear_and_free_semaphores(list(self.sems.allocated().values()))
        self.nc.all_engine_barrier()

    tile.TileContext._drain_and_barrier = _drain_and_barrier
    tile.TileContext._drain_chunk_patched = True


def _split_excess_waits(nc, maxw=1, maxw_other=None):
    """Walrus rejects instructions carrying more than ~1 sem wait. Move excess
    waits onto same-engine NoOps inserted immediately before the instruction
    (engine streams execute in bb order, so semantics are preserved)."""
    from bass_rust import InstNoOp

    ctr = 0
    for f in nc.m.functions:
        for bb in f.blocks:
            new_insts = []
            for inst in bb.instructions:
                si = inst.sync_info
                waits = list(si.on_wait) if si and si.on_wait else []
                lim = maxw
                if maxw_other is not None and str(inst.engine) != 'EngineType.PE':
                    lim = maxw_other
                maxw_eff = lim
                if len(waits) > maxw_eff:
                    head, rest = waits[:-maxw_eff], waits[-maxw_eff:]
                    for k in range(0, len(head), maxw_eff):
                        ctr += 1
                        new_insts.append(InstNoOp(
                            name=f"waitnop_{ctr}",
                            engine=inst.engine,
                            sync_info=mybir.SyncInfo(
                                on_wait=head[k:k + maxw_eff], on_update=[]),
                        ))
                    inst.sync_info = mybir.SyncInfo(on_wait=rest, on_update=si.on_update)
                new_insts.append(inst)
            bb.instructions = new_insts
    return ctr


def _build_program(split_waits=True):
    _patch_drain_chunking()
    nc = bass.Bass()

    XDT = F8 if USE_FP8_QKV else BF16
    xT_d = nc.declare_dram_parameter("xT", [C, T], XDT, isOutput=False)
    wq_d = nc.declare_dram_parameter("wqT", [C, GC], XDT, isOutput=False)
    wk_d = nc.declare_dram_parameter("wkT", [C, GC], XDT, isOutput=False)
    wv_d = nc.declare_dram_parameter("wvT", [C, GC], XDT, isOutput=False)
    wp_d = nc.declare_dram_parameter("wpT", [GC, C], BF16, isOutput=False)
    out_d = nc.declare_dram_parameter("outp", [T, C], F32, isOutput=True)
    exp_scale = 0.125 / (WSCALE * WSCALE) if USE_FP8_QKV else 0.125
    v_unscale = 1.0 / WSCALE if USE_FP8_QKV else None

    from contextlib import ExitStack

    with tile.TileContext(nc) as tc, ExitStack() as stack:
        ep = stack.enter_context

        cpool = ep(tc.tile_pool(name="const", bufs=1))
        # tril[k, q] = 1 where q >= k else 0: multiplied into the diagonal
        # p blocks post-exp on the otherwise-idle gpsimd engine
        tril = cpool.tile([128, 128], BF16)
        nc.gpsimd.memset(tril[:, :], 1.0)
        nc.gpsimd.affine_select(
            out=tril[:, :], in_=tril[:, :],
            compare_op=mybir.AluOpType.is_ge, fill=0.0, base=0,
            pattern=[[1, 128]], channel_multiplier=-1,
        )
        # identity for PE transpose
        ident = cpool.tile([128, 128], BF16)
        nc.gpsimd.memset(ident[:, :], 1.0)
        nc.gpsimd.affine_select(
            out=ident[:, :], in_=ident[:, :],
            compare_op=mybir.AluOpType.is_ge, fill=0.0, base=0,
            pattern=[[1, 128]], channel_multiplier=-1,
        )
        nc.gpsimd.affine_select(
            out=ident[:, :], in_=ident[:, :],
            compare_op=mybir.AluOpType.is_ge, fill=0.0, base=0,
            pattern=[[-1, 128]], channel_multiplier=1,
        )

        # persistent activation buffers
        big = ep(tc.tile_pool(name="big", bufs=1))
        qT = big.tile([128, NPAIR, T], BF16)    # [d-of-pair, pair, t]
        kT = big.tile([128, NPAIR, T], BF16)
        v = big.tile([128, NQC, HPC, D + 1], BF16)  # [t%128, t-chunk, head, d|1]
        avT = big.tile([128, NPAIR, T], BF16)   # [d-of-pair(gc-chunk), pair, t]
        nc.vector.memset(v[:, :, :, D:D + 1], 1.0)
        # warm the ACT exp table before the critical path
        warm = cpool.tile([128, 1], F32)
        nc.vector.memset(warm[:, :], 0.0)
        nc.scalar.activation(warm[:, :], warm[:, :], AF.Exp, scale=0.0)

        # weights
        wpool = ep(tc.tile_pool(name="w", bufs=1))
        wq = wpool.tile([128, CCH, GC], XDT)
        wk = wpool.tile([128, CCH, GC], XDT)
        wv = wpool.tile([128, CCH, GC], XDT)
        wp = wpool.tile([128, GC // 128, C], BF16)

        xpool = ep(tc.tile_pool(name="xs", bufs=1))
        ppool = ep(tc.tile_pool(name="pt", bufs=7))
        avsb_pool = ep(tc.tile_pool(name="avsb", bufs=3))
        ob_pool = ep(tc.tile_pool(name="ob", bufs=3))
        rc_pool = ep(tc.tile_pool(name="rc", bufs=3))
        ps_s = ep(tc.tile_pool(name="ps_s", bufs=2, space="PSUM"))
        ps_av_pool = ep(tc.tile_pool(name="ps_av", bufs=1, space="PSUM"))
        ps_m = ep(tc.tile_pool(name="ps_m", bufs=2, space="PSUM"))

        # av accumulators: one bank per head-parity (concurrent accumulation
        # groups must not share a PSUM bank: start=True zeroes the whole bank),
        # with two pair-slots per bank used strictly sequentially.
        av_a = ps_av_pool.tile([128, 2, D + 1], F32)   # h2 = 0, [slot, d|denom]
        av_b = ps_av_pool.tile([128, 2, D + 1], F32)   # h2 = 1
        av_banks = (av_a, av_b)

        xs = xpool.tile([128, NSTRIP, CCH, 512], XDT)

        def emit_qkv_mm(pq, s, kind, o, j):
            """Micro-step j (0..7) of one 128-row projection group."""
            w_sb = {"q": wq, "k": wk, "v": wv}[kind]
            if USE_FP8_QKV:
                half, g = j // 4, j % 4
                if kind == "v":
                    lhsT = xs[:, s, 2 * g:2 * g + 2,
                              128 * o + 64 * half:128 * o + 64 * half + 64]
                    rhs = wv[:, 2 * g:2 * g + 2, :]
                else:
                    lhsT = w_sb[:, 2 * g:2 * g + 2,
                                128 * o + 64 * half:128 * o + 64 * half + 64]
                    rhs = xs[:, s, 2 * g:2 * g + 2, :]
                nc.tensor.matmul(pq[64 * half:64 * half + 64, :], lhsT, rhs,
                                 start=(g == 0), stop=(g == 3),
                                 perf_mode=DR, skip_group_check=True)
            else:
                c = j
                if kind == "v":
                    nc.tensor.matmul(pq[:, :], xs[:, s, c, 128 * o:128 * (o + 1)],
                                     wv[:, c, :], start=(c == 0),
                                     stop=(c == CCH - 1), skip_group_check=True)
                else:
                    nc.tensor.matmul(pq[:, :], w_sb[:, c, 128 * o:128 * (o + 1)],
                                     xs[:, s, c, :], start=(c == 0),
                                     stop=(c == CCH - 1), skip_group_check=True)

        def emit_qkv_fin(pq, s, kind, o):
            if kind == "v":
                nc.vector.tensor_copy(
                    v[:, 4 * s + o, :, 0:D],
                    pq[:, :].rearrange("p (h d) -> p h d", h=HPC))
            else:
                dst = qT if kind == "q" else kT
                nc.vector.tensor_copy(dst[:, o, 512 * s:512 * (s + 1)], pq[:, :])

        def emit_qkv_group(s, kind, o):
            pq = ps_m.tile([128, 512], F32, tag="m")
            for j in range(8):
                emit_qkv_mm(pq, s, kind, o, j)
            emit_qkv_fin(pq, s, kind, o)

        # ---- startup: fine-grained first DMAs, V first so the PE can start
        # after one 128-col x quarter + half of Wv instead of the whole strip.
        def dma_x_piece(s, lo, hi):
            nc.sync.dma_start(
                xs[:, s, :, lo:hi],
                xT_d[:, 512 * s + lo:512 * s + hi].rearrange(
                    "(c p) t -> p c t", p=128))

        wv_r = wv_d[:, :].rearrange("(c p) o -> p c o", p=128)
        wq_r = wq_d[:, :].rearrange("(c p) o -> p c o", p=128)
        nc.sync.dma_start(wv[:, 0:2, :], wv_r[:, 0:2, :])
        dma_x_piece(0, 0, 128)
        nc.sync.dma_start(wv[:, 2:5, :], wv_r[:, 2:5, :])
        nc.sync.dma_start(wv[:, 5:8, :], wv_r[:, 5:8, :])
        dma_x_piece(0, 128, 256)
        nc.sync.dma_start(wq[:, 0:2, :], wq_r[:, 0:2, :])
        dma_x_piece(0, 256, 384)
        nc.sync.dma_start(wq[:, 2:4, :], wq_r[:, 2:4, :])
        dma_x_piece(0, 384, 512)
        nc.sync.dma_start(wq[:, 4:6, :], wq_r[:, 4:6, :])
        nc.sync.dma_start(wq[:, 6:8, :], wq_r[:, 6:8, :])
        nc.sync.dma_start(wk[:, :, :], wk_d[:, :].rearrange("(c p) o -> p c o", p=128))
        for o in range(4):
            emit_qkv_group(0, "v", o)
        for s in range(1, NSTRIP):
            dma_x_piece(s, 0, 512)
        nc.sync.dma_start(wp[:, :, :], wp_d[:, :].rearrange("(g p) o -> p g o", p=128))
        for kind in ("q", "k"):
            for o in range(4):
                emit_qkv_group(0, kind, o)

        # ---------------- attention + projection pipeline ----------------
        def emit_scores_exp(qc, pair, g, w, split_exp=False):
            sps = ps_s.tile([128, 2, 4, 128], F32, tag="s")
            for h2 in range(2):
                pb = 64 * h2
                for i in range(w):
                    kc = 4 * g + i
                    nc.tensor.matmul(
                        sps[:, h2, i, :],
                        kT[pb:pb + 64, pair, 128 * kc:128 * (kc + 1)],
                        qT[pb:pb + 64, pair, 128 * qc:128 * (qc + 1)],
                        start=True, stop=True)
            p = ppool.tile([128, 2, 4, 128], BF16, tag="p")
            if split_exp:
                # per-parity exp halves shorten the dependence chain at the
                # kernel tail (av h2=0 starts while h2=1 still exponentiates)
                for h2 in range(2):
                    nc.scalar.activation(p[:, h2, 0:w, :], sps[:, h2, 0:w, :],
                                         AF.Exp, scale=exp_scale)
            else:
                nc.scalar.activation(p[:, :, 0:w, :], sps[:, :, 0:w, :],
                                     AF.Exp, scale=exp_scale)
            # tril-mask the diagonal block (kc == qc) post-exp, on gpsimd
            if 4 * g <= qc < 4 * (g + 1):
                i = qc - 4 * g
                for h2 in range(2):
                    nc.gpsimd.tensor_tensor(p[:, h2, i, :], p[:, h2, i, :],
                                            tril[:, :], op=MULT)
            return p

        def emit_av(qc, pair, g, w, p):
            slot = pair % 2
            nkc = qc + 1
            for h2 in range(2):
                head = 2 * pair + h2
                for i in range(w):
                    kc = 4 * g + i
                    nc.tensor.matmul(
                        av_banks[h2][:, slot, :], p[:, h2, i, :],
                        v[:, kc, head, :],
                        start=(kc == 0), stop=(kc == nkc - 1),
                        skip_group_check=True)

        def emit_normalize(qc, pair):
            slot = pair % 2
            av_sb = avsb_pool.tile([128, 128], BF16, tag="avsb")
            rc = rc_pool.tile([128, 2], F32, tag="rc")
            for h2 in range(2):
                nc.vector.reciprocal(rc[:, h2:h2 + 1], av_banks[h2][:, slot, D:D + 1])
                if v_unscale is None:
                    nc.vector.tensor_scalar(
                        out=av_sb[:, 64 * h2:64 * (h2 + 1)],
                        in0=av_banks[h2][:, slot, 0:D],
                        scalar1=rc[:, h2:h2 + 1], scalar2=None, op0=MULT)
                else:
                    nc.vector.tensor_scalar(
                        out=av_sb[:, 64 * h2:64 * (h2 + 1)],
                        in0=av_banks[h2][:, slot, 0:D],
                        scalar1=rc[:, h2:h2 + 1], scalar2=v_unscale,
                        op0=MULT, op1=MULT)
            return av_sb

        def emit_transpose(qc, pair, av_sb):
            # pos_of_qc is defined with the item stream below; calls happen
            # only after it exists
            if pos_of_qc[qc] < NQC - 2:
                # xbar DMA transpose: frees PE + DVE; ~2.5us issue latency is
                # hidden by the 2-pair transpose lag
                nc.sync.dma_start_transpose(
                    avT[:, pair, 128 * qc:128 * (qc + 1)], av_sb[:, :])
            else:
                # tail q-chunks stay on the PE to keep the flush chain short
                tp = ps_m.tile([128, 128], BF16, tag="m")
                nc.tensor.transpose(tp[:, :], av_sb[:, :], ident[:, :])
                nc.vector.tensor_copy(avT[:, pair, 128 * qc:128 * (qc + 1)], tp[:, :])

        def emit_proj(tt, split_dma=False):
            ob = ob_pool.tile([128, C], F32, tag="ob")
            for o2 in range(2):
                po = ps_m.tile([128, 512], F32, tag="m")
                for c4 in range(GC // 128):
                    nc.tensor.matmul(po[:, :], avT[:, c4, 128 * tt:128 * (tt + 1)],
                                     wp[:, c4, 512 * o2:512 * (o2 + 1)],
                                     start=(c4 == 0), stop=(c4 == GC // 128 - 1))
                nc.vector.tensor_copy(ob[:, 512 * o2:512 * (o2 + 1)], po[:, :])
                if split_dma:
                    nc.sync.dma_start(
                        out_d[128 * tt:128 * (tt + 1), 512 * o2:512 * (o2 + 1)],
                        ob[:, 512 * o2:512 * (o2 + 1)])
            if not split_dma:
                nc.sync.dma_start(out_d[128 * tt:128 * (tt + 1), :], ob[:, :])

        # item stream with a 1-item software pipeline (av lags scores by one)
        qc_order = list(range(NQC))
        pos_of_qc = {qc: i for i, qc in enumerate(qc_order)}
        items = []
        first_item_of_qc = {}
        for qc in qc_order:
            nkc = qc + 1
            ngroups = (nkc + 3) // 4
            first_item_of_qc[qc] = len(items)
            for pair in range(NPAIR):
                for g in range(ngroups):
                    w = min(4, nkc - 4 * g)
                    items.append((qc, pair, g, w))
        first_item_of_qc[NQC] = len(items)

        # strip fillers at single-matmul granularity: the per-item PE deficit
        # vs ACT exp is ~450 ns, so whole 1.7 us QKV groups cause run-ahead
        # jitter against the 2-deep sps rotation. A credit model doles out
        # individual accumulation matmuls (213 ns each) to keep the PE stream
        # dense and smooth; strip s must complete before qc = 4s.
        MM = 107.0 if USE_FP8_QKV else 213.0
        MM2 = 213.0          # proj matmuls are always bf16 ap-512
        strip_micro = []     # (strip, fn, pe_cost, kind: 'mm0'|'mm'|'fin')

        def make_strip_micro():
            for s in range(1, NSTRIP):
                for kind in ("q", "k", "v"):
                    for o in range(4):
                        pq_cell = []
                        for j in range(8):
                            def mm(s=s, kind=kind, o=o, j=j, pq_cell=pq_cell):
                                if j == 0:
                                    pq_cell.append(
                                        ps_m.tile([128, 512], F32, tag="m", name="pq"))
                                emit_qkv_mm(pq_cell[-1], s, kind, o, j)
                            strip_micro.append(
                                (s, mm, MM, "mm0" if j == 0 else "mm"))

                        def fin(s=s, kind=kind, o=o, pq_cell=pq_cell):
                            emit_qkv_fin(pq_cell[-1], s, kind, o)
                        strip_micro.append((s, fin, 0.0, "fin"))

        make_strip_micro()

        def queue_proj_micro(tt):
            # append one output-projection t-chunk as paced micro-ops (o2
            # halves of 4 accumulating matmuls each + copy/store epilogue);
            # tagged 99 so force_strip never touches it
            ob_cell = []
            for o2 in range(2):
                po_cell = []
                for c4 in range(GC // 128):
                    def mm(tt=tt, o2=o2, c4=c4, po_cell=po_cell, ob_cell=ob_cell):
                        if c4 == 0:
                            if o2 == 0:
                                ob_cell.append(
                                    ob_pool.tile([128, C], F32, tag="ob", name="ob"))
                            po_cell.append(
                                ps_m.tile([128, 512], F32, tag="m", name="po"))
                        nc.tensor.matmul(
                            po_cell[-1][:, :], avT[:, c4, 128 * tt:128 * (tt + 1)],
                            wp[:, c4, 512 * o2:512 * (o2 + 1)],
                            start=(c4 == 0), stop=(c4 == GC // 128 - 1),
                            skip_group_check=True)
                    strip_micro.append(
                        (99, mm, MM2, "mm0" if c4 == 0 else "mm"))

                def fin(tt=tt, o2=o2, po_cell=po_cell, ob_cell=ob_cell):
                    ob = ob_cell[-1]
                    nc.vector.tensor_copy(ob[:, 512 * o2:512 * (o2 + 1)],
                                          po_cell[-1][:, :])
                    nc.sync.dma_start(
                        out_d[128 * tt:128 * (tt + 1), 512 * o2:512 * (o2 + 1)],
                        ob[:, 512 * o2:512 * (o2 + 1)])
                strip_micro.append((99, fin, 0.0, "fin"))

        strip_ptr = [0]
        credit = [0.0]

        def _emit_next():
            s, fn, cost, k = strip_micro[strip_ptr[0]]
            fn()
            credit[0] -= cost
            strip_ptr[0] += 1

        def pace():
            while strip_ptr[0] < len(strip_micro):
                s, fn, cost, k = strip_micro[strip_ptr[0]]
                if credit[0] < cost and cost > 0:
                    break
                _emit_next()

        def close_open_group():
            # a mid-accumulation strip group holds a ps_m slot; any other
            # ps_m allocation while it is open can deadlock the in-order PE
            # queue on slot reuse, so finish the group first
            while strip_ptr[0] < len(strip_micro) and \
                    strip_micro[strip_ptr[0]][3] != "mm0":
                _emit_next()

        def force_strip(s_done):
            # everything belonging to strips <= s_done must be emitted now
            while strip_ptr[0] < len(strip_micro) and \
                    strip_micro[strip_ptr[0]][0] <= s_done:
                _emit_next()

        AV_LAG = 3
        prevs = []            # [(qc, pair, g, w, p)] av software-pipeline lag
        pending_norm = {}     # (qc, pair) -> av_sb awaiting transpose

        transposed_count = {}
        next_proj_tt = [0]

        def drain_transposes(upto_idx):
            # transpose every pending pair whose sequence index is <= upto_idx
            for key in sorted(pending_norm, key=lambda k: pos_of_qc[k[0]] * NPAIR + k[1]):
                if pos_of_qc[key[0]] * NPAIR + key[1] <= upto_idx:
                    emit_transpose(key[0], key[1], pending_norm.pop(key))
                    credit[0] -= 53.0
                    transposed_count[key[0]] = transposed_count.get(key[0], 0) + 1
            # once a q-chunk is fully transposed its projection becomes
            # pace-able filler (tt = qc_order[-1] stays in the flush)
            while next_proj_tt[0] != qc_order[-1] and \
                    transposed_count.get(next_proj_tt[0], 0) == NPAIR:
                queue_proj_micro(next_proj_tt[0])
                next_proj_tt[0] += 1

        last_pair_key = None
        for idx, (qc, pair, g, w) in enumerate(items):
            if qc % 4 == 0 and pair == 0 and g == 0 and qc // 4 >= 1:
                force_strip(qc // 4)
                credit[0] = max(credit[0], -2000.0)
            if (qc, pair) != last_pair_key:
                drain_transposes(pos_of_qc[qc] * NPAIR + pair - 2)
                last_pair_key = (qc, pair)
            # per-item ACT-vs-PE deficit feeds the filler credit
            credit[0] += (213.3 * w + 245.0) - (160.8 * w)
            pace()

            p = emit_scores_exp(qc, pair, g, w,
                                split_exp=(idx == len(items) - 1))
            if len(prevs) == AV_LAG:
                pqc, ppair, pg, pw, pp_ = prevs.pop(0)
                emit_av(pqc, ppair, pg, pw, pp_)
                if 4 * (pg + 1) >= pqc + 1:  # last group of that pair
                    pending_norm[(pqc, ppair)] = emit_normalize(pqc, ppair)
            prevs.append((qc, pair, g, w, p))

        # flush: drain any remaining paced filler, then interleave the last
        # proj's pair-3-independent matmuls with the final transpose chain
        # (alloc order keeps the 2-slot ps_m rotation deadlock-free)
        for pqc, ppair, pg, pw, pp_ in prevs:
            emit_av(pqc, ppair, pg, pw, pp_)
            if 4 * (pg + 1) >= pqc + 1:
                pending_norm[(pqc, ppair)] = emit_normalize(pqc, ppair)
        while strip_ptr[0] < len(strip_micro):
            _emit_next()
        key = (qc_order[-1], 2)
        if key in pending_norm:
            emit_transpose(key[0], key[1], pending_norm.pop(key))
        tt = qc_order[-1]
        ob = ob_pool.tile([128, C], F32, tag="ob")
        po0 = ps_m.tile([128, 512], F32, tag="m")
        for c4 in range(3):
            nc.tensor.matmul(po0[:, :], avT[:, c4, 128 * tt:128 * (tt + 1)],
                             wp[:, c4, 0:512], start=(c4 == 0), stop=False,
                             skip_group_check=True)
        drain_transposes(NQC * NPAIR)
        nc.tensor.matmul(po0[:, :], avT[:, 3, 128 * tt:128 * (tt + 1)],
                         wp[:, 3, 0:512], start=False, stop=True,
                         skip_group_check=True)
        nc.vector.tensor_copy(ob[:, 0:512], po0[:, :])
        nc.sync.dma_start(out_d[128 * tt:128 * (tt + 1), 0:512], ob[:, 0:512])
        po1 = ps_m.tile([128, 512], F32, tag="m")
        for c4 in range(GC // 128):
            nc.tensor.matmul(po1[:, :], avT[:, c4, 128 * tt:128 * (tt + 1)],
                             wp[:, c4, 512:1024], start=(c4 == 0),
                             stop=(c4 == GC // 128 - 1), skip_group_check=True)
        # quarter-granular epilogue so the store pipeline drains sooner
        for qtr in range(2):
            lo, hi = 512 + 256 * qtr, 512 + 256 * (qtr + 1)
            nc.vector.tensor_copy(ob[:, lo:hi], po1[:, lo - 512:hi - 512])
            nc.sync.dma_start(out_d[128 * tt:128 * (tt + 1), lo:hi], ob[:, lo:hi])

    if split_waits:
        _split_excess_waits(nc)
    return nc


def _get_program():
    global _PROGRAM
    if _PROGRAM is None:
        _PROGRAM = _build_program()
    return _PROGRAM


def _make_in_maps(x, Wk, Wq, Wv, Wp):
    import ml_dtypes
    bf = ml_dtypes.bfloat16
    xdt = ml_dtypes.float8_e4m3 if USE_FP8_QKV else bf
    ws = WSCALE if USE_FP8_QKV else 1.0
    x = np.asarray(x, dtype=np.float32)
    Wk = np.asarray(Wk, dtype=np.float32)
    Wq = np.asarray(Wq, dtype=np.float32)
    Wv = np.asarray(Wv, dtype=np.float32)
    Wp = np.asarray(Wp, dtype=np.float32)
    in_maps = []
    for core in range(8):
        b, g = core // GROUPS, core % GROUPS
        rows = slice(GC * g, GC * (g + 1))
        in_maps.append({
            "xT": np.ascontiguousarray(x[b].T).astype(xdt),           # [C, T]
            "wqT": np.ascontiguousarray(Wq[rows, :].T * ws).astype(xdt),
            "wkT": np.ascontiguousarray(Wk[rows, :].T * ws).astype(xdt),
            "wvT": np.ascontiguousarray(Wv[rows, :].T * ws).astype(xdt),
            "wpT": np.ascontiguousarray(Wp[:, rows].T).astype(bf),    # [GC, C]
        })
    return in_maps


def run(x, Wk, Wq, Wv, Wp, bp, trace=False, **spmd_kwargs):
    nc = _get_program()
    in_maps = _make_in_maps(x, Wk, Wq, Wv, Wp)
    res = run_bass_kernel_spmd(nc, in_maps, list(range(8)), trace=trace, **spmd_kwargs)
    bp = np.asarray(bp, dtype=np.float32)
    out = np.empty((B, T, C), dtype=np.float32)
    for b in range(B):
        out[b] = res.results[GROUPS * b]["outp"] + res.results[GROUPS * b + 1]["outp"] + bp
    return out, res


def kernel(x, Wk, Wq, Wv, Wp, bp):
    out, _ = run(x, Wk, Wq, Wv, Wp, bp)
    return out


# revision 66
# speedup vs baseline: 1.6603x; 1.0152x over previous
"""Multi-head causal self-attention (B=4, T=2048, C=1024, 16 heads) on 8 trn2 cores.

Sharding: data-parallel over batch (4) x tensor-parallel over heads (2 groups of 8).
Core m handles batch m//2, head group m%2. Host pre-transposes x and the weights
(bf16); the output projection partial sums are pair-reduced on host (+bias).

Per-core pipeline (all matmul operands bf16, fp32 PSUM accumulate):
  qT[d2,t], kT[d2,t] = W @ x^T per head-pair; v[t,h,d] = x @ Wv^T  (+ ones col)
  per (q-chunk, head-pair): scores^T block [k,q] = kT-chunk-slice.T @ qT-slice
  p = exp(0.125*scores) (one ACT op per 2-head 4-chunk group, bf16 out), tril
  mask on the diagonal block by bf16 multiply on gpsimd, then TRANSPOSED AV:
  av[q, d|1] += p_block.T @ v~ -- ap=65 per block instead of 512 (the matmul
  cost is the moving size; the p block rides the free stationary port), with
  the ones column of v~ producing the softmax denominators in column d.
  normalize = DVE reciprocal + per-partition tensor_scalar mult -> av_sb [q,d2]
  transpose to avT [d2, t] via xbar DMA (PE-transpose for the last chunks);
  out[t, :] = avT-chunks.T @ Wp^T, staged to SBUF, DMA to DRAM.

Scheduling: the attention item stream (q-chunk, head-pair, 4-k-chunk group) is
ACT-heavier than PE per item, so QKV strip matmuls (strip s due before qc=4s)
and later the output projections are doled out as single-matmul micro-ops by a
credit pacer that keeps the in-order PE stream dense; AV runs 3 items behind
scores so it never waits out the exp latency; PSUM banks: 4 scores (2-bank
pair tiles x2), 2 av accumulators (one per head parity; concurrent groups must
not share a bank since start=True zeroes it), 2 rotating qkv/proj/transpose.
"""

import numpy as np

import concourse.bass as bass
import concourse.mybir as mybir
import concourse.tile as tile
from concourse.bass_utils import run_bass_kernel_spmd

F32 = mybir.dt.float32
BF16 = mybir.dt.bfloat16
F8 = mybir.dt.float8e4
AF = mybir.ActivationFunctionType
MULT = mybir.AluOpType.mult
DR = mybir.MatmulPerfMode.DoubleRow

# QKV projections in fp8e4m3 DoubleRow (2x PE throughput). Host pre-scales the
# QKV weights by WSCALE to dodge fp8 subnormals; compensated via the exp scale
# (q,k) and the normalize epilogue (v).
USE_FP8_QKV = False
WSCALE = 32.0

B, T, C = 4, 2048, 1024
HEADS, D = 16, 64
GROUPS = 2                  # head groups (tensor parallel)
HPC = HEADS // GROUPS       # heads per core = 8
NPAIR = HPC // 2            # head pairs per core = 4
GC = HPC * D                # group channel width = 512
NQC = T // 128              # q chunks (and k chunks) = 16
CCH = C // 128              # contraction chunks = 8
NSTRIP = T // 512           # qkv t strips = 4

_PROGRAM = None


def _patch_drain_chunking():
    """The axon walrus build rejects instructions with >~4 sem waits; Tile's
    kernel-tail drain waits on every live semaphore at once. Split it into a
    chain of drains with <=2 waits each."""
    from bass_rust import VectorClock, ScopedClock

    if getattr(tile.TileContext, "_drain_chunk_patched", False):
        return

    def _drain_and_barrier(self, tick_clock, wait_clock):
        gc_vec = list(tick_clock.global_clock)
        nz = [i for i, t in enumerate(gc_vec) if t > 0]
        CHUNK = 2
        for k in range(0, len(nz), CHUNK):
            keep = set(nz[k:k + CHUNK])
            partial = [gc_vec[i] if i in keep else 0 for i in range(len(gc_vec))]
            d = self.nc.sync.drain()
            wait_clock.add_sem_waits(d.ins, ScopedClock({None: VectorClock(partial)}))
        self.nc.all_engine_barrier()
        assert self.sems is not None
        popped = self.nc._tile_sem_poison_stack.pop()
        assert popped is self._sem_poison
        self.nc.clear_and_free_semaphores(list(self.sems.allocated().values()))
        self.nc.all_engine_barrier()

    tile.TileContext._drain_and_barrier = _drain_and_barrier
    tile.TileContext._drain_chunk_patched = True


def _split_excess_waits(nc, maxw=1, maxw_other=None):
    """Walrus rejects instructions carrying more than ~1 sem wait. Move excess
    waits onto same-engine NoOps inserted immediately before the instruction
    (engine streams execute in bb order, so semantics are preserved)."""
    from bass_rust import InstNoOp

    ctr = 0
    for f in nc.m.functions:
        for bb in f.blocks:
            new_insts = []
            for inst in bb.instructions:
                si = inst.sync_info
                waits = list(si.on_wait) if si and si.on_wait else []
                lim = maxw
                if maxw_other is not None and str(inst.engine) != 'EngineType.PE':
                    lim = maxw_other
                maxw_eff = lim
                if len(waits) > maxw_eff:
                    head, rest = waits[:-maxw_eff], waits[-maxw_eff:]
                    for k in range(0, len(head), maxw_eff):
                        ctr += 1
                        new_insts.append(InstNoOp(
                            name=f"waitnop_{ctr}",
                            engine=inst.engine,
                            sync_info=mybir.SyncInfo(
                                on_wait=head[k:k + maxw_eff], on_update=[]),
                        ))
                    inst.sync_info = mybir.SyncInfo(on_wait=rest, on_update=si.on_update)
                new_insts.append(inst)
            bb.instructions = new_insts
    return ctr


def _build_program(split_waits=True):
    _patch_drain_chunking()
    nc = bass.Bass()

    XDT = F8 if USE_FP8_QKV else BF16
    xT_d = nc.declare_dram_parameter("xT", [C, T], XDT, isOutput=False)
    wq_d = nc.declare_dram_parameter("wqT", [C, GC], XDT, isOutput=False)
    wk_d = nc.declare_dram_parameter("wkT", [C, GC], XDT, isOutput=False)
    wv_d = nc.declare_dram_parameter("wvT", [C, GC], XDT, isOutput=False)
    wp_d = nc.declare_dram_parameter("wpT", [GC, C], BF16, isOutput=False)
    out_d = nc.declare_dram_parameter("outp", [T, C], F32, isOutput=True)
    exp_scale = 0.125 / (WSCALE * WSCALE) if USE_FP8_QKV else 0.125
    v_unscale = 1.0 / WSCALE if USE_FP8_QKV else None

    from contextlib import ExitStack

    with tile.TileContext(nc) as tc, ExitStack() as stack:
        ep = stack.enter_context

        cpool = ep(tc.tile_pool(name="const", bufs=1))
        # tril[k, q] = 1 where q >= k else 0: multiplied into the diagonal
        # p blocks post-exp on the otherwise-idle gpsimd engine
        tril = cpool.tile([128, 128], BF16)
        nc.gpsimd.memset(tril[:, :], 1.0)
        nc.gpsimd.affine_select(
            out=tril[:, :], in_=tril[:, :],
            compare_op=mybir.AluOpType.is_ge, fill=0.0, base=0,
            pattern=[[1, 128]], channel_multiplier=-1,
        )
        # identity for PE transpose
        ident = cpool.tile([128, 128], BF16)
        nc.gpsimd.memset(ident[:, :], 1.0)
        nc.gpsimd.affine_select(
            out=ident[:, :], in_=ident[:, :],
            compare_op=mybir.AluOpType.is_ge, fill=0.0, base=0,
            pattern=[[1, 128]], channel_multiplier=-1,
        )
        nc.gpsimd.affine_select(
            out=ident[:, :], in_=ident[:, :],
            compare_op=mybir.AluOpType.is_ge, fill=0.0, base=0,
            pattern=[[-1, 128]], channel_multiplier=1,
        )

        # persistent activation buffers
        big = ep(tc.tile_pool(name="big", bufs=1))
        qT = big.tile([128, NPAIR, T], BF16)    # [d-of-pair, pair, t]
        kT = big.tile([128, NPAIR, T], BF16)
        v = big.tile([128, NQC, HPC, D + 1], BF16)  # [t%128, t-chunk, head, d|1]
        avT = big.tile([128, NPAIR, T], BF16)   # [d-of-pair(gc-chunk), pair, t]
        nc.vector.memset(v[:, :, :, D:D + 1], 1.0)
        # warm the ACT exp table before the critical path
        warm = cpool.tile([128, 1], F32)
        nc.vector.memset(warm[:, :], 0.0)
        nc.scalar.activation(warm[:, :], warm[:, :], AF.Exp, scale=0.0)

        # weights
        wpool = ep(tc.tile_pool(name="w", bufs=1))
        wq = wpool.tile([128, CCH, GC], XDT)
        wk = wpool.tile([128, CCH, GC], XDT)
        wv = wpool.tile([128, CCH, GC], XDT)
        wp = wpool.tile([128, GC // 128, C], BF16)

        xpool = ep(tc.tile_pool(name="xs", bufs=1))
        ppool = ep(tc.tile_pool(name="pt", bufs=7))
        avsb_pool = ep(tc.tile_pool(name="avsb", bufs=3))
        ob_pool = ep(tc.tile_pool(name="ob", bufs=3))
        rc_pool = ep(tc.tile_pool(name="rc", bufs=3))
        ps_s = ep(tc.tile_pool(name="ps_s", bufs=2, space="PSUM"))
        ps_av_pool = ep(tc.tile_pool(name="ps_av", bufs=1, space="PSUM"))
        ps_m = ep(tc.tile_pool(name="ps_m", bufs=2, space="PSUM"))

        # av accumulators: one bank per head-parity (concurrent accumulation
        # groups must not share a PSUM bank: start=True zeroes the whole bank),
        # with two pair-slots per bank used strictly sequentially.
        av_a = ps_av_pool.tile([128, 2, D + 1], F32)   # h2 = 0, [slot, d|denom]
        av_b = ps_av_pool.tile([128, 2, D + 1], F32)   # h2 = 1
        av_banks = (av_a, av_b)

        xs = xpool.tile([128, NSTRIP, CCH, 512], XDT)

        def emit_qkv_mm(pq, s, kind, o, j):
            """Micro-step j (0..7) of one 128-row projection group."""
            w_sb = {"q": wq, "k": wk, "v": wv}[kind]
            if USE_FP8_QKV:
                half, g = j // 4, j % 4
                if kind == "v":
                    lhsT = xs[:, s, 2 * g:2 * g + 2,
                              128 * o + 64 * half:128 * o + 64 * half + 64]
                    rhs = wv[:, 2 * g:2 * g + 2, :]
                else:
                    lhsT = w_sb[:, 2 * g:2 * g + 2,
                                128 * o + 64 * half:128 * o + 64 * half + 64]
                    rhs = xs[:, s, 2 * g:2 * g + 2, :]
                nc.tensor.matmul(pq[64 * half:64 * half + 64, :], lhsT, rhs,
                                 start=(g == 0), stop=(g == 3),
                                 perf_mode=DR, skip_group_check=True)
            else:
                c = j
                if kind == "v":
                    nc.tensor.matmul(pq[:, :], xs[:, s, c, 128 * o:128 * (o + 1)],
                                     wv[:, c, :], start=(c == 0),
                                     stop=(c == CCH - 1), skip_group_check=True)
                else:
                    nc.tensor.matmul(pq[:, :], w_sb[:, c, 128 * o:128 * (o + 1)],
                                     xs[:, s, c, :], start=(c == 0),
                                     stop=(c == CCH - 1), skip_group_check=True)

        def emit_qkv_fin(pq, s, kind, o):
            if kind == "v":
                nc.vector.tensor_copy(
                    v[:, 4 * s + o, :, 0:D],
                    pq[:, :].rearrange("p (h d) -> p h d", h=HPC))
            else:
                dst = qT if kind == "q" else kT
                nc.vector.tensor_copy(dst[:, o, 512 * s:512 * (s + 1)], pq[:, :])

        def emit_qkv_group(s, kind, o):
            pq = ps_m.tile([128, 512], F32, tag="m")
            for j in range(8):
                emit_qkv_mm(pq, s, kind, o, j)
            emit_qkv_fin(pq, s, kind, o)

        # ---- startup: fine-grained first DMAs, V first so the PE can start
        # after one 128-col x quarter + half of Wv instead of the whole strip.
        def dma_x_piece(s, lo, hi):
            nc.sync.dma_start(
                xs[:, s, :, lo:hi],
                xT_d[:, 512 * s + lo:512 * s + hi].rearrange(
                    "(c p) t -> p c t", p=128))

        wv_r = wv_d[:, :].rearrange("(c p) o -> p c o", p=128)
        wq_r = wq_d[:, :].rearrange("(c p) o -> p c o", p=128)
        nc.sync.dma_start(wv[:, 0:2, :], wv_r[:, 0:2, :])
        dma_x_piece(0, 0, 128)
        nc.sync.dma_start(wv[:, 2:5, :], wv_r[:, 2:5, :])
        nc.sync.dma_start(wv[:, 5:8, :], wv_r[:, 5:8, :])
        dma_x_piece(0, 128, 256)
        nc.sync.dma_start(wq[:, 0:2, :], wq_r[:, 0:2, :])
        dma_x_piece(0, 256, 384)
        nc.sync.dma_start(wq[:, 2:4, :], wq_r[:, 2:4, :])
        dma_x_piece(0, 384, 512)
        nc.sync.dma_start(wq[:, 4:6, :], wq_r[:, 4:6, :])
        nc.sync.dma_start(wq[:, 6:8, :], wq_r[:, 6:8, :])
        nc.sync.dma_start(wk[:, :, :], wk_d[:, :].rearrange("(c p) o -> p c o", p=128))
        for o in range(4):
            emit_qkv_group(0, "v", o)
        for s in range(1, NSTRIP):
            dma_x_piece(s, 0, 512)
        nc.sync.dma_start(wp[:, :, :], wp_d[:, :].rearrange("(g p) o -> p g o", p=128))
        for kind in ("q", "k"):
            for o in range(4):
                emit_qkv_group(0, kind, o)

        # ---------------- attention + projection pipeline ----------------
        def emit_scores_exp(qc, pair, g, w, split_exp=False):
            sps = ps_s.tile([128, 2, 4, 128], F32, tag="s")
            for h2 in range(2):
                pb = 64 * h2
                for i in range(w):
                    kc = 4 * g + i
                    nc.tensor.matmul(
                        sps[:, h2, i, :],
                        kT[pb:pb + 64, pair, 128 * kc:128 * (kc + 1)],
                        qT[pb:pb + 64, pair, 128 * qc:128 * (qc + 1)],
                        start=True, stop=True)
            p = ppool.tile([128, 2, 4, 128], BF16, tag="p")
            if split_exp:
                # per-parity exp halves shorten the dependence chain at the
                # kernel tail (av h2=0 starts while h2=1 still exponentiates)
                for h2 in range(2):
                    nc.scalar.activation(p[:, h2, 0:w, :], sps[:, h2, 0:w, :],
                                         AF.Exp, scale=exp_scale)
            else:
                nc.scalar.activation(p[:, :, 0:w, :], sps[:, :, 0:w, :],
                                     AF.Exp, scale=exp_scale)
            # tril-mask the diagonal block (kc == qc) post-exp, on gpsimd
            if 4 * g <= qc < 4 * (g + 1):
                i = qc - 4 * g
                for h2 in range(2):
                    nc.gpsimd.tensor_tensor(p[:, h2, i, :], p[:, h2, i, :],
                                            tril[:, :], op=MULT)
            return p

        def emit_av(qc, pair, g, w, p):
            slot = pair % 2
            nkc = qc + 1
            for h2 in range(2):
                head = 2 * pair + h2
                for i in range(w):
                    kc = 4 * g + i
                    nc.tensor.matmul(
                        av_banks[h2][:, slot, :], p[:, h2, i, :],
                        v[:, kc, head, :],
                        start=(kc == 0), stop=(kc == nkc - 1),
                        skip_group_check=True)

        def emit_normalize(qc, pair):
            slot = pair % 2
            av_sb = avsb_pool.tile([128, 128], BF16, tag="avsb")
            rc = rc_pool.tile([128, 2], F32, tag="rc")
            for h2 in range(2):
                nc.vector.reciprocal(rc[:, h2:h2 + 1], av_banks[h2][:, slot, D:D + 1])
                if v_unscale is None:
                    nc.vector.tensor_scalar(
                        out=av_sb[:, 64 * h2:64 * (h2 + 1)],
                        in0=av_banks[h2][:, slot, 0:D],
                        scalar1=rc[:, h2:h2 + 1], scalar2=None, op0=MULT)
                else:
                    nc.vector.tensor_scalar(
                        out=av_sb[:, 64 * h2:64 * (h2 + 1)],
                        in0=av_banks[h2][:, slot, 0:D],
                        scalar1=rc[:, h2:h2 + 1], scalar2=v_unscale,
                        op0=MULT, op1=MULT)
            return av_sb

        def emit_transpose(qc, pair, av_sb):
            # pos_of_qc is defined with the item stream below; calls happen
            # only after it exists
            if pos_of_qc[qc] < NQC - 2:
                # xbar DMA transpose: frees PE + DVE; ~2.5us issue latency is
                # hidden by the 2-pair transpose lag
                nc.sync.dma_start_transpose(
                    avT[:, pair, 128 * qc:128 * (qc + 1)], av_sb[:, :])
            else:
                # tail q-chunks stay on the PE to keep the flush chain short
                tp = ps_m.tile([128, 128], BF16, tag="m")
                nc.tensor.transpose(tp[:, :], av_sb[:, :], ident[:, :])
                nc.vector.tensor_copy(avT[:, pair, 128 * qc:128 * (qc + 1)], tp[:, :])

        # item stream with a 1-item software pipeline (av lags scores by one)
        qc_order = list(range(NQC))
        pos_of_qc = {qc: i for i, qc in enumerate(qc_order)}
        items = []
        first_item_of_qc = {}
        for qc in qc_order:
            nkc = qc + 1
            ngroups = (nkc + 3) // 4
            first_item_of_qc[qc] = len(items)
            for pair in range(NPAIR):
                for g in range(ngroups):
                    w = min(4, nkc - 4 * g)
                    items.append((qc, pair, g, w))
        first_item_of_qc[NQC] = len(items)

        # strip fillers at single-matmul granularity: the per-item PE deficit
        # vs ACT exp is ~450 ns, so whole 1.7 us QKV groups cause run-ahead
        # jitter against the 2-deep sps rotation. A credit model doles out
        # individual accumulation matmuls (213 ns each) to keep the PE stream
        # dense and smooth; strip s must complete before qc = 4s.
        MM = 107.0 if USE_FP8_QKV else 213.0
        MM2 = 213.0          # proj matmuls are always bf16 ap-512
        strip_micro = []     # (strip, fn, pe_cost, kind: 'mm0'|'mm'|'fin')

        def make_strip_micro():
            for s in range(1, NSTRIP):
                for kind in ("q", "k", "v"):
                    for o in range(4):
                        pq_cell = []
                        for j in range(8):
                            def mm(s=s, kind=kind, o=o, j=j, pq_cell=pq_cell):
                                if j == 0:
                                    pq_cell.append(
                                        ps_m.tile([128, 512], F32, tag="m", name="pq"))
                                emit_qkv_mm(pq_cell[-1], s, kind, o, j)
                            strip_micro.append(
                                (s, mm, MM, "mm0" if j == 0 else "mm"))

                        def fin(s=s, kind=kind, o=o, pq_cell=pq_cell):
                            emit_qkv_fin(pq_cell[-1], s, kind, o)
                        strip_micro.append((s, fin, 0.0, "fin"))

        make_strip_micro()

        def queue_proj_micro(tt):
            # append one output-projection t-chunk as paced micro-ops (o2
            # halves of 4 accumulating matmuls each + copy/store epilogue);
            # tagged 99 so force_strip never touches it
            ob_cell = []
            for o2 in range(2):
                po_cell = []
                for c4 in range(GC // 128):
                    def mm(tt=tt, o2=o2, c4=c4, po_cell=po_cell, ob_cell=ob_cell):
                        if c4 == 0:
                            if o2 == 0:
                                ob_cell.append(
                                    ob_pool.tile([128, C], F32, tag="ob", name="ob"))
                            po_cell.append(
                                ps_m.tile([128, 512], F32, tag="m", name="po"))
                        nc.tensor.matmul(
                            po_cell[-1][:, :], avT[:, c4, 128 * tt:128 * (tt + 1)],
                            wp[:, c4, 512 * o2:512 * (o2 + 1)],
                            start=(c4 == 0), stop=(c4 == GC // 128 - 1),
                            skip_group_check=True)
                    strip_micro.append(
                        (99, mm, MM2, "mm0" if c4 == 0 else "mm"))

                def fin(tt=tt, o2=o2, po_cell=po_cell, ob_cell=ob_cell):
                    ob = ob_cell[-1]
                    nc.vector.tensor_copy(ob[:, 512 * o2:512 * (o2 + 1)],
                                          po_cell[-1][:, :])
                    nc.sync.dma_start(
                        out_d[128 * tt:128 * (tt + 1), 512 * o2:512 * (o2 + 1)],
                        ob[:, 512 * o2:512 * (o2 + 1)])
                strip_micro.append((99, fin, 0.0, "fin"))

        strip_ptr = [0]
        credit = [0.0]

        def _emit_next():
            s, fn, cost, k = strip_micro[strip_ptr[0]]
            fn()
            credit[0] -= cost
            strip_ptr[0] += 1

        def pace():
            while strip_ptr[0] < len(strip_micro):
                s, fn, cost, k = strip_micro[strip_ptr[0]]
                if credit[0] < cost and cost > 0:
                    break
                _emit_next()

        def close_open_group():
            # a mid-accumulation strip group holds a ps_m slot; any other
            # ps_m allocation while it is open can deadlock the in-order PE
            # queue on slot reuse, so finish the group first
            while strip_ptr[0] < len(strip_micro) and \
                    strip_micro[strip_ptr[0]][3] != "mm0":
                _emit_next()

        def force_strip(s_done):
            # everything belonging to strips <= s_done must be emitted now
            while strip_ptr[0] < len(strip_micro) and \
                    strip_micro[strip_ptr[0]][0] <= s_done:
                _emit_next()

        AV_LAG = 3
        prevs = []            # [(qc, pair, g, w, p)] av software-pipeline lag
        pending_norm = {}     # (qc, pair) -> av_sb awaiting transpose

        transposed_count = {}
        next_proj_tt = [0]

        def drain_transposes(upto_idx):
            # transpose every pending pair whose sequence index is <= upto_idx
            for key in sorted(pending_norm, key=lambda k: pos_of_qc[k[0]] * NPAIR + k[1]):
                if pos_of_qc[key[0]] * NPAIR + key[1] <= upto_idx:
                    emit_transpose(key[0], key[1], pending_norm.pop(key))
                    credit[0] -= 53.0
                    transposed_count[key[0]] = transposed_count.get(key[0], 0) + 1
            # once a q-chunk is fully transposed its projection becomes
            # pace-able filler (tt = qc_order[-1] stays in the flush)
            while next_proj_tt[0] != qc_order[-1] and \
                    transposed_count.get(next_proj_tt[0], 0) == NPAIR:
                queue_proj_micro(next_proj_tt[0])
                next_proj_tt[0] += 1

        last_pair_key = None
        for idx, (qc, pair, g, w) in enumerate(items):
            if qc % 4 == 0 and pair == 0 and g == 0 and qc // 4 >= 1:
                force_strip(qc // 4)
                credit[0] = max(credit[0], -2000.0)
            if (qc, pair) != last_pair_key:
                drain_transposes(pos_of_qc[qc] * NPAIR + pair - 2)
                last_pair_key = (qc, pair)
            # per-item ACT-vs-PE deficit feeds the filler credit
            credit[0] += (213.3 * w + 245.0) - (160.8 * w)
            pace()

            p = emit_scores_exp(qc, pair, g, w,
                                split_exp=(idx == len(items) - 1))
            if len(prevs) == AV_LAG:
                pqc, ppair, pg, pw, pp_ = prevs.pop(0)
                emit_av(pqc, ppair, pg, pw, pp_)
                if 4 * (pg + 1) >= pqc + 1:  # last group of that pair
                    pending_norm[(pqc, ppair)] = emit_normalize(pqc, ppair)
            prevs.append((qc, pair, g, w, p))

        # flush: drain any remaining paced filler, then interleave the last
        # proj's pair-3-independent matmuls with the final transpose chain
        # (alloc order keeps the 2-slot ps_m rotation deadlock-free)
        for pqc, ppair, pg, pw, pp_ in prevs:
            emit_av(pqc, ppair, pg, pw, pp_)
            if 4 * (pg + 1) >= pqc + 1:
                pending_norm[(pqc, ppair)] = emit_normalize(pqc, ppair)
        while strip_ptr[0] < len(strip_micro):
            _emit_next()
        key = (qc_order[-1], 2)
        if key in pending_norm:
            emit_transpose(key[0], key[1], pending_norm.pop(key))
        tt = qc_order[-1]
        ob = ob_pool.tile([128, C], F32, tag="ob")
        po0 = ps_m.tile([128, 512], F32, tag="m")
        for c4 in range(3):
            nc.tensor.matmul(po0[:, :], avT[:, c4, 128 * tt:128 * (tt + 1)],
                             wp[:, c4, 0:512], start=(c4 == 0), stop=False,
                             skip_group_check=True)
        drain_transposes(NQC * NPAIR)
        nc.tensor.matmul(po0[:, :], avT[:, 3, 128 * tt:128 * (tt + 1)],
                         wp[:, 3, 0:512], start=False, stop=True,
                         skip_group_check=True)
        nc.vector.tensor_copy(ob[:, 0:512], po0[:, :])
        nc.sync.dma_start(out_d[128 * tt:128 * (tt + 1), 0:512], ob[:, 0:512])
        po1 = ps_m.tile([128, 512], F32, tag="m")
        for c4 in range(GC // 128):
            nc.tensor.matmul(po1[:, :], avT[:, c4, 128 * tt:128 * (tt + 1)],
                             wp[:, c4, 512:1024], start=(c4 == 0),
                             stop=(c4 == GC // 128 - 1), skip_group_check=True)
        # quarter-granular epilogue so the store pipeline drains sooner
        for qtr in range(2):
            lo, hi = 512 + 256 * qtr, 512 + 256 * (qtr + 1)
            nc.vector.tensor_copy(ob[:, lo:hi], po1[:, lo - 512:hi - 512])
            nc.sync.dma_start(out_d[128 * tt:128 * (tt + 1), lo:hi], ob[:, lo:hi])

    if split_waits:
        _split_excess_waits(nc)
    return nc


def _get_program():
    global _PROGRAM
    if _PROGRAM is None:
        _PROGRAM = _build_program()
    return _PROGRAM


def _make_in_maps(x, Wk, Wq, Wv, Wp):
    import ml_dtypes
    bf = ml_dtypes.bfloat16
    xdt = ml_dtypes.float8_e4m3 if USE_FP8_QKV else bf
    ws = WSCALE if USE_FP8_QKV else 1.0
    x = np.asarray(x, dtype=np.float32)
    Wk = np.asarray(Wk, dtype=np.float32)
    Wq = np.asarray(Wq, dtype=np.float32)
    Wv = np.asarray(Wv, dtype=np.float32)
    Wp = np.asarray(Wp, dtype=np.float32)
    in_maps = []
    for core in range(8):
        b, g = core // GROUPS, core % GROUPS
        rows = slice(GC * g, GC * (g + 1))
        in_maps.append({
            "xT": np.ascontiguousarray(x[b].T).astype(xdt),           # [C, T]
            "wqT": np.ascontiguousarray(Wq[rows, :].T * ws).astype(xdt),
            "wkT": np.ascontiguousarray(Wk[rows, :].T * ws).astype(xdt),
            "wvT": np.ascontiguousarray(Wv[rows, :].T * ws).astype(xdt),
            "wpT": np.ascontiguousarray(Wp[:, rows].T).astype(bf),    # [GC, C]
        })
    return in_maps


def run(x, Wk, Wq, Wv, Wp, bp, trace=False, **spmd_kwargs):
    nc = _get_program()
    in_maps = _make_in_maps(x, Wk, Wq, Wv, Wp)
    res = run_bass_kernel_spmd(nc, in_maps, list(range(8)), trace=trace, **spmd_kwargs)
    bp = np.asarray(bp, dtype=np.float32)
    out = np.empty((B, T, C), dtype=np.float32)
    for b in range(B):
        out[b] = res.results[GROUPS * b]["outp"] + res.results[GROUPS * b + 1]["outp"] + bp
    return out, res


def kernel(x, Wk, Wq, Wv, Wp, bp):
    out, _ = run(x, Wk, Wq, Wv, Wp, bp)
    return out


# revision 72
# speedup vs baseline: 1.6634x; 1.0018x over previous
"""Multi-head causal self-attention (B=4, T=2048, C=1024, 16 heads) on 8 trn2 cores.

Sharding: data-parallel over batch (4) x tensor-parallel over heads (2 groups of 8).
Core m handles batch m//2, head group m%2. Host pre-transposes x and the weights
(bf16); the output projection partial sums are pair-reduced on host (+bias).

Per-core pipeline (all matmul operands bf16, fp32 PSUM accumulate):
  qT[d2,t], kT[d2,t] = W @ x^T per head-pair; v[t,h,d] = x @ Wv^T  (+ ones col)
  per (q-chunk, head-pair): scores^T block [k,q] = kT-chunk-slice.T @ qT-slice
  p = exp(0.125*scores) (one ACT op per 2-head 4-chunk group, bf16 out), tril
  mask on the diagonal block by bf16 multiply on gpsimd, then TRANSPOSED AV:
  av[q, d|1] += p_block.T @ v~ -- ap=65 per block instead of 512 (the matmul
  cost is the moving size; the p block rides the free stationary port), with
  the ones column of v~ producing the softmax denominators in column d.
  normalize = DVE reciprocal + per-partition tensor_scalar mult -> av_sb [q,d2]
  transpose to avT [d2, t] via xbar DMA (PE-transpose for the last chunks);
  out[t, :] = avT-chunks.T @ Wp^T, staged to SBUF, DMA to DRAM.

Scheduling: the attention item stream (q-chunk, head-pair, 4-k-chunk group) is
ACT-heavier than PE per item, so QKV strip matmuls (strip s due before qc=4s)
and later the output projections are doled out as single-matmul micro-ops by a
credit pacer that keeps the in-order PE stream dense; AV runs 3 items behind
scores so it never waits out the exp latency; PSUM banks: 4 scores (2-bank
pair tiles x2), 2 av accumulators (one per head parity; concurrent groups must
not share a bank since start=True zeroes it), 2 rotating qkv/proj/transpose.
"""

import numpy as np

import concourse.bass as bass
import concourse.mybir as mybir
import concourse.tile as tile
from concourse.bass_utils import run_bass_kernel_spmd

F32 = mybir.dt.float32
BF16 = mybir.dt.bfloat16
F8 = mybir.dt.float8e4
AF = mybir.ActivationFunctionType
MULT = mybir.AluOpType.mult
DR = mybir.MatmulPerfMode.DoubleRow

# QKV projections in fp8e4m3 DoubleRow (2x PE throughput). Host pre-scales the
# QKV weights by WSCALE to dodge fp8 subnormals; compensated via the exp scale
# (q,k) and the normalize epilogue (v).
USE_FP8_QKV = False
WSCALE = 32.0

B, T, C = 4, 2048, 1024
HEADS, D = 16, 64
GROUPS = 2                  # head groups (tensor parallel)
HPC = HEADS // GROUPS       # heads per core = 8
NPAIR = HPC // 2            # head pairs per core = 4
GC = HPC * D                # group channel width = 512
NQC = T // 128              # q chunks (and k chunks) = 16
CCH = C // 128              # contraction chunks = 8
NSTRIP = T // 512           # qkv t strips = 4

_PROGRAM = None


def _patch_drain_chunking():
    """The axon walrus build rejects instructions with >~4 sem waits; Tile's
    kernel-tail drain waits on every live semaphore at once. Split it into a
    chain of drains with <=2 waits each."""
    from bass_rust import VectorClock, ScopedClock

    if getattr(tile.TileContext, "_drain_chunk_patched", False):
        return

    def _drain_and_barrier(self, tick_clock, wait_clock):
        gc_vec = list(tick_clock.global_clock)
        nz = [i for i, t in enumerate(gc_vec) if t > 0]
        CHUNK = 2
        for k in range(0, len(nz), CHUNK):
            keep = set(nz[k:k + CHUNK])
            partial = [gc_vec[i] if i in keep else 0 for i in range(len(gc_vec))]
            d = self.nc.sync.drain()
            wait_clock.add_sem_waits(d.ins, ScopedClock({None: VectorClock(partial)}))
        self.nc.all_engine_barrier()
        assert self.sems is not None
        popped = self.nc._tile_sem_poison_stack.pop()
        assert popped is self._sem_poison
        self.nc.clear_and_free_semaphores(list(self.sems.allocated().values()))
        self.nc.all_engine_barrier()

    tile.TileContext._drain_and_barrier = _drain_and_barrier
    tile.TileContext._drain_chunk_patched = True


def _split_excess_waits(nc, maxw=1, maxw_other=None):
    """Walrus rejects instructions carrying more than ~1 sem wait. Move excess
    waits onto same-engine NoOps inserted immediately before the instruction
    (engine streams execute in bb order, so semantics are preserved)."""
    from bass_rust import InstNoOp

    ctr = 0
    for f in nc.m.functions:
        for bb in f.blocks:
            new_insts = []
            for inst in bb.instructions:
                si = inst.sync_info
                waits = list(si.on_wait) if si and si.on_wait else []
                lim = maxw
                if maxw_other is not None and str(inst.engine) != 'EngineType.PE':
                    lim = maxw_other
                maxw_eff = lim
                if len(waits) > maxw_eff:
                    head, rest = waits[:-maxw_eff], waits[-maxw_eff:]
                    for k in range(0, len(head), maxw_eff):
                        ctr += 1
                        new_insts.append(InstNoOp(
                            name=f"waitnop_{ctr}",
                            engine=inst.engine,
                            sync_info=mybir.SyncInfo(
                                on_wait=head[k:k + maxw_eff], on_update=[]),
                        ))
                    inst.sync_info = mybir.SyncInfo(on_wait=rest, on_update=si.on_update)
                new_insts.append(inst)
            bb.instructions = new_insts
    return ctr


def _build_program(split_waits=True):
    _patch_drain_chunking()
    nc = bass.Bass()

    XDT = F8 if USE_FP8_QKV else BF16
    xT_d = nc.declare_dram_parameter("xT", [C, T], XDT, isOutput=False)
    wq_d = nc.declare_dram_parameter("wqT", [C, GC], XDT, isOutput=False)
    wk_d = nc.declare_dram_parameter("wkT", [C, GC], XDT, isOutput=False)
    wv_d = nc.declare_dram_parameter("wvT", [C, GC], XDT, isOutput=False)
    wp_d = nc.declare_dram_parameter("wpT", [GC, C], BF16, isOutput=False)
    out_d = nc.declare_dram_parameter("outp", [T, C], F32, isOutput=True)
    exp_scale = 0.125 / (WSCALE * WSCALE) if USE_FP8_QKV else 0.125
    v_unscale = 1.0 / WSCALE if USE_FP8_QKV else None

    from contextlib import ExitStack

    with tile.TileContext(nc) as tc, ExitStack() as stack:
        ep = stack.enter_context

        cpool = ep(tc.tile_pool(name="const", bufs=1))
        # tril[k, q] = 1 where q >= k else 0: multiplied into the diagonal
        # p blocks post-exp on the otherwise-idle gpsimd engine
        tril = cpool.tile([128, 128], BF16)
        nc.gpsimd.memset(tril[:, :], 1.0)
        nc.gpsimd.affine_select(
            out=tril[:, :], in_=tril[:, :],
            compare_op=mybir.AluOpType.is_ge, fill=0.0, base=0,
            pattern=[[1, 128]], channel_multiplier=-1,
        )
        # identity for PE transpose
        ident = cpool.tile([128, 128], BF16)
        nc.gpsimd.memset(ident[:, :], 1.0)
        nc.gpsimd.affine_select(
            out=ident[:, :], in_=ident[:, :],
            compare_op=mybir.AluOpType.is_ge, fill=0.0, base=0,
            pattern=[[1, 128]], channel_multiplier=-1,
        )
        nc.gpsimd.affine_select(
            out=ident[:, :], in_=ident[:, :],
            compare_op=mybir.AluOpType.is_ge, fill=0.0, base=0,
            pattern=[[-1, 128]], channel_multiplier=1,
        )

        # persistent activation buffers
        big = ep(tc.tile_pool(name="big", bufs=1))
        qT = big.tile([128, NPAIR, T], BF16)    # [d-of-pair, pair, t]
        kT = big.tile([128, NPAIR, T], BF16)
        v = big.tile([128, NQC, HPC, D + 1], BF16)  # [t%128, t-chunk, head, d|1]
        avT = big.tile([128, NPAIR, T], BF16)   # [d-of-pair(gc-chunk), pair, t]
        nc.vector.memset(v[:, :, :, D:D + 1], 1.0)
        # warm the ACT exp table before the critical path
        warm = cpool.tile([128, 1], F32)
        nc.vector.memset(warm[:, :], 0.0)
        nc.scalar.activation(warm[:, :], warm[:, :], AF.Exp, scale=0.0)

        # weights
        wpool = ep(tc.tile_pool(name="w", bufs=1))
        wq = wpool.tile([128, CCH, GC], XDT)
        wk = wpool.tile([128, CCH, GC], XDT)
        wv = wpool.tile([128, CCH, GC], XDT)
        wp = wpool.tile([128, GC // 128, C], BF16)

        xpool = ep(tc.tile_pool(name="xs", bufs=1))
        ppool = ep(tc.tile_pool(name="pt", bufs=7))
        avsb_pool = ep(tc.tile_pool(name="avsb", bufs=3))
        ob_pool = ep(tc.tile_pool(name="ob", bufs=3))
        rc_pool = ep(tc.tile_pool(name="rc", bufs=3))
        ps_s = ep(tc.tile_pool(name="ps_s", bufs=2, space="PSUM"))
        ps_av_pool = ep(tc.tile_pool(name="ps_av", bufs=1, space="PSUM"))
        ps_m = ep(tc.tile_pool(name="ps_m", bufs=2, space="PSUM"))

        # av accumulators: one bank per head-parity (concurrent accumulation
        # groups must not share a PSUM bank: start=True zeroes the whole bank),
        # with two pair-slots per bank used strictly sequentially.
        av_a = ps_av_pool.tile([128, 2, D + 1], F32)   # h2 = 0, [slot, d|denom]
        av_b = ps_av_pool.tile([128, 2, D + 1], F32)   # h2 = 1
        av_banks = (av_a, av_b)

        xs = xpool.tile([128, NSTRIP, CCH, 512], XDT)

        def emit_qkv_mm(pq, s, kind, o, j):
            """Micro-step j (0..7) of one 128-row projection group."""
            w_sb = {"q": wq, "k": wk, "v": wv}[kind]
            if USE_FP8_QKV:
                half, g = j // 4, j % 4
                if kind == "v":
                    lhsT = xs[:, s, 2 * g:2 * g + 2,
                              128 * o + 64 * half:128 * o + 64 * half + 64]
                    rhs = wv[:, 2 * g:2 * g + 2, :]
                else:
                    lhsT = w_sb[:, 2 * g:2 * g + 2,
                                128 * o + 64 * half:128 * o + 64 * half + 64]
                    rhs = xs[:, s, 2 * g:2 * g + 2, :]
                nc.tensor.matmul(pq[64 * half:64 * half + 64, :], lhsT, rhs,
                                 start=(g == 0), stop=(g == 3),
                                 perf_mode=DR, skip_group_check=True)
            else:
                c = j
                if kind == "v":
                    nc.tensor.matmul(pq[:, :], xs[:, s, c, 128 * o:128 * (o + 1)],
                                     wv[:, c, :], start=(c == 0),
                                     stop=(c == CCH - 1), skip_group_check=True)
                else:
                    nc.tensor.matmul(pq[:, :], w_sb[:, c, 128 * o:128 * (o + 1)],
                                     xs[:, s, c, :], start=(c == 0),
                                     stop=(c == CCH - 1), skip_group_check=True)

        def emit_qkv_fin(pq, s, kind, o):
            if kind == "v":
                nc.vector.tensor_copy(
                    v[:, 4 * s + o, :, 0:D],
                    pq[:, :].rearrange("p (h d) -> p h d", h=HPC))
            else:
                dst = qT if kind == "q" else kT
                nc.vector.tensor_copy(dst[:, o, 512 * s:512 * (s + 1)], pq[:, :])

        def emit_qkv_group(s, kind, o):
            pq = ps_m.tile([128, 512], F32, tag="m")
            for j in range(8):
                emit_qkv_mm(pq, s, kind, o, j)
            emit_qkv_fin(pq, s, kind, o)

        # ---- startup: fine-grained first DMAs, V first so the PE can start
        # after one 128-col x quarter + half of Wv instead of the whole strip.
        def dma_x_piece(s, lo, hi):
            nc.sync.dma_start(
                xs[:, s, :, lo:hi],
                xT_d[:, 512 * s + lo:512 * s + hi].rearrange(
                    "(c p) t -> p c t", p=128))

        wv_r = wv_d[:, :].rearrange("(c p) o -> p c o", p=128)
        wq_r = wq_d[:, :].rearrange("(c p) o -> p c o", p=128)
        nc.sync.dma_start(wv[:, 0:2, :], wv_r[:, 0:2, :])
        dma_x_piece(0, 0, 128)
        nc.sync.dma_start(wv[:, 2:5, :], wv_r[:, 2:5, :])
        nc.sync.dma_start(wv[:, 5:8, :], wv_r[:, 5:8, :])
        dma_x_piece(0, 128, 256)
        nc.sync.dma_start(wq[:, 0:2, :], wq_r[:, 0:2, :])
        dma_x_piece(0, 256, 384)
        nc.sync.dma_start(wq[:, 2:4, :], wq_r[:, 2:4, :])
        dma_x_piece(0, 384, 512)
        nc.sync.dma_start(wq[:, 4:6, :], wq_r[:, 4:6, :])
        nc.sync.dma_start(wq[:, 6:8, :], wq_r[:, 6:8, :])
        nc.sync.dma_start(wk[:, :, :], wk_d[:, :].rearrange("(c p) o -> p c o", p=128))
        for o in range(4):
            emit_qkv_group(0, "v", o)
        for s in range(1, NSTRIP):
            dma_x_piece(s, 0, 512)
        nc.sync.dma_start(wp[:, :, :], wp_d[:, :].rearrange("(g p) o -> p g o", p=128))
        for kind in ("q", "k"):
            for o in range(4):
                emit_qkv_group(0, kind, o)

        # ---------------- attention + projection pipeline ----------------
        def emit_scores_exp(qc, pair, g, w, split_exp=False):
            sps = ps_s.tile([128, 2, 4, 128], F32, tag="s")
            for h2 in range(2):
                pb = 64 * h2
                for i in range(w):
                    kc = 4 * g + i
                    nc.tensor.matmul(
                        sps[:, h2, i, :],
                        kT[pb:pb + 64, pair, 128 * kc:128 * (kc + 1)],
                        qT[pb:pb + 64, pair, 128 * qc:128 * (qc + 1)],
                        start=True, stop=True)
            p = ppool.tile([128, 2, 4, 128], BF16, tag="p")
            if split_exp:
                # per-parity exp halves shorten the dependence chain at the
                # kernel tail (av h2=0 starts while h2=1 still exponentiates)
                for h2 in range(2):
                    nc.scalar.activation(p[:, h2, 0:w, :], sps[:, h2, 0:w, :],
                                         AF.Exp, scale=exp_scale)
            else:
                nc.scalar.activation(p[:, :, 0:w, :], sps[:, :, 0:w, :],
                                     AF.Exp, scale=exp_scale)
            # tril-mask the diagonal block (kc == qc) post-exp, on gpsimd
            if 4 * g <= qc < 4 * (g + 1):
                i = qc - 4 * g
                for h2 in range(2):
                    nc.gpsimd.tensor_tensor(p[:, h2, i, :], p[:, h2, i, :],
                                            tril[:, :], op=MULT)
            return p

        def emit_av(qc, pair, g, w, p):
            slot = pair % 2
            nkc = qc + 1
            for h2 in range(2):
                head = 2 * pair + h2
                for i in range(w):
                    kc = 4 * g + i
                    nc.tensor.matmul(
                        av_banks[h2][:, slot, :], p[:, h2, i, :],
                        v[:, kc, head, :],
                        start=(kc == 0), stop=(kc == nkc - 1),
                        skip_group_check=True)

        def emit_normalize(qc, pair):
            slot = pair % 2
            av_sb = avsb_pool.tile([128, 128], BF16, tag="avsb")
            rc = rc_pool.tile([128, 2], F32, tag="rc")
            for h2 in range(2):
                nc.vector.reciprocal(rc[:, h2:h2 + 1], av_banks[h2][:, slot, D:D + 1])
                if v_unscale is None:
                    nc.vector.tensor_scalar(
                        out=av_sb[:, 64 * h2:64 * (h2 + 1)],
                        in0=av_banks[h2][:, slot, 0:D],
                        scalar1=rc[:, h2:h2 + 1], scalar2=None, op0=MULT)
                else:
                    nc.vector.tensor_scalar(
                        out=av_sb[:, 64 * h2:64 * (h2 + 1)],
                        in0=av_banks[h2][:, slot, 0:D],
                        scalar1=rc[:, h2:h2 + 1], scalar2=v_unscale,
                        op0=MULT, op1=MULT)
            return av_sb

        def emit_transpose(qc, pair, av_sb):
            # pos_of_qc is defined with the item stream below; calls happen
            # only after it exists
            if pos_of_qc[qc] < NQC - 2:
                # xbar DMA transpose: frees PE + DVE; ~2.5us issue latency is
                # hidden by the 2-pair transpose lag
                nc.sync.dma_start_transpose(
                    avT[:, pair, 128 * qc:128 * (qc + 1)], av_sb[:, :])
            else:
                # tail q-chunks stay on the PE to keep the flush chain short
                tp = ps_m.tile([128, 128], BF16, tag="m")
                nc.tensor.transpose(tp[:, :], av_sb[:, :], ident[:, :])
                nc.vector.tensor_copy(avT[:, pair, 128 * qc:128 * (qc + 1)], tp[:, :])

        # item stream with a 1-item software pipeline (av lags scores by one)
        qc_order = list(range(NQC))
        pos_of_qc = {qc: i for i, qc in enumerate(qc_order)}
        items = []
        first_item_of_qc = {}
        for qc in qc_order:
            nkc = qc + 1
            ngroups = (nkc + 3) // 4
            first_item_of_qc[qc] = len(items)
            for pair in range(NPAIR):
                for g in range(ngroups):
                    w = min(4, nkc - 4 * g)
                    items.append((qc, pair, g, w))
        first_item_of_qc[NQC] = len(items)

        # strip fillers at single-matmul granularity: the per-item PE deficit
        # vs ACT exp is ~450 ns, so whole 1.7 us QKV groups cause run-ahead
        # jitter against the 2-deep sps rotation. A credit model doles out
        # individual accumulation matmuls (213 ns each) to keep the PE stream
        # dense and smooth; strip s must complete before qc = 4s.
        MM = 107.0 if USE_FP8_QKV else 213.0
        MM2 = 213.0          # proj matmuls are always bf16 ap-512
        strip_micro = []     # (strip, fn, pe_cost, kind: 'mm0'|'mm'|'fin')

        def make_strip_micro():
            for s in range(1, NSTRIP):
                for kind in ("q", "k", "v"):
                    for o in range(4):
                        pq_cell = []
                        for j in range(8):
                            def mm(s=s, kind=kind, o=o, j=j, pq_cell=pq_cell):
                                if j == 0:
                                    pq_cell.append(
                                        ps_m.tile([128, 512], F32, tag="m", name="pq"))
                                emit_qkv_mm(pq_cell[-1], s, kind, o, j)
                            strip_micro.append(
                                (s, mm, MM, "mm0" if j == 0 else "mm"))

                        def fin(s=s, kind=kind, o=o, pq_cell=pq_cell):
                            emit_qkv_fin(pq_cell[-1], s, kind, o)
                        strip_micro.append((s, fin, 0.0, "fin"))

        make_strip_micro()

        def queue_proj_micro(tt):
            # append one output-projection t-chunk as paced micro-ops (o2
            # halves of 4 accumulating matmuls each + copy/store epilogue);
            # tagged 99 so force_strip never touches it
            ob_cell = []
            for o2 in range(2):
                po_cell = []
                for c4 in range(GC // 128):
                    def mm(tt=tt, o2=o2, c4=c4, po_cell=po_cell, ob_cell=ob_cell):
                        if c4 == 0:
                            if o2 == 0:
                                ob_cell.append(
                                    ob_pool.tile([128, C], F32, tag="ob", name="ob"))
                            po_cell.append(
                                ps_m.tile([128, 512], F32, tag="m", name="po"))
                        nc.tensor.matmul(
                            po_cell[-1][:, :], avT[:, c4, 128 * tt:128 * (tt + 1)],
                            wp[:, c4, 512 * o2:512 * (o2 + 1)],
                            start=(c4 == 0), stop=(c4 == GC // 128 - 1),
                            skip_group_check=True)
                    strip_micro.append(
                        (99, mm, MM2, "mm0" if c4 == 0 else "mm"))

                def fin(tt=tt, o2=o2, po_cell=po_cell, ob_cell=ob_cell):
                    ob = ob_cell[-1]
                    nc.vector.tensor_copy(ob[:, 512 * o2:512 * (o2 + 1)],
                                          po_cell[-1][:, :])
                    nc.sync.dma_start(
                        out_d[128 * tt:128 * (tt + 1), 512 * o2:512 * (o2 + 1)],
                        ob[:, 512 * o2:512 * (o2 + 1)])
                strip_micro.append((99, fin, 0.0, "fin"))

        strip_ptr = [0]
        credit = [0.0]

        def _emit_next():
            s, fn, cost, k = strip_micro[strip_ptr[0]]
            fn()
            credit[0] -= cost
            strip_ptr[0] += 1

        def pace():
            while strip_ptr[0] < len(strip_micro):
                s, fn, cost, k = strip_micro[strip_ptr[0]]
                if credit[0] < cost and cost > 0:
                    break
                _emit_next()

        def close_open_group():
            # a mid-accumulation strip group holds a ps_m slot; any other
            # ps_m allocation while it is open can deadlock the in-order PE
            # queue on slot reuse, so finish the group first
            while strip_ptr[0] < len(strip_micro) and \
                    strip_micro[strip_ptr[0]][3] != "mm0":
                _emit_next()

        def force_strip(s_done):
            # everything belonging to strips <= s_done must be emitted now
            while strip_ptr[0] < len(strip_micro) and \
                    strip_micro[strip_ptr[0]][0] <= s_done:
                _emit_next()

        AV_LAG = 3
        prevs = []            # [(qc, pair, g, w, p)] av software-pipeline lag
        pending_norm = {}     # (qc, pair) -> av_sb awaiting transpose

        transposed_count = {}
        next_proj_tt = [0]

        def drain_transposes(upto_idx):
            # transpose every pending pair whose sequence index is <= upto_idx
            for key in sorted(pending_norm, key=lambda k: pos_of_qc[k[0]] * NPAIR + k[1]):
                if pos_of_qc[key[0]] * NPAIR + key[1] <= upto_idx:
                    emit_transpose(key[0], key[1], pending_norm.pop(key))
                    credit[0] -= 53.0
                    transposed_count[key[0]] = transposed_count.get(key[0], 0) + 1
            # once a q-chunk is fully transposed its projection becomes
            # pace-able filler (tt = qc_order[-1] stays in the flush)
            while next_proj_tt[0] != qc_order[-1] and \
                    transposed_count.get(next_proj_tt[0], 0) == NPAIR:
                queue_proj_micro(next_proj_tt[0])
                next_proj_tt[0] += 1

        last_pair_key = None
        for idx, (qc, pair, g, w) in enumerate(items):
            if qc % 4 == 0 and pair == 0 and g == 0 and qc // 4 >= 1:
                force_strip(qc // 4)
                credit[0] = max(credit[0], -2000.0)
            if (qc, pair) != last_pair_key:
                drain_transposes(pos_of_qc[qc] * NPAIR + pair - 2)
                last_pair_key = (qc, pair)
            # per-item ACT-vs-PE deficit feeds the filler credit
            credit[0] += (213.3 * w + 265.0) - (160.8 * w)
            pace()

            p = emit_scores_exp(qc, pair, g, w,
                                split_exp=(idx == len(items) - 1))
            if len(prevs) == AV_LAG:
                pqc, ppair, pg, pw, pp_ = prevs.pop(0)
                emit_av(pqc, ppair, pg, pw, pp_)
                if 4 * (pg + 1) >= pqc + 1:  # last group of that pair
                    pending_norm[(pqc, ppair)] = emit_normalize(pqc, ppair)
            prevs.append((qc, pair, g, w, p))

        # flush: drain any remaining paced filler, then interleave the last
        # proj's pair-3-independent matmuls with the final transpose chain
        # (alloc order keeps the 2-slot ps_m rotation deadlock-free)
        for pqc, ppair, pg, pw, pp_ in prevs:
            emit_av(pqc, ppair, pg, pw, pp_)
            if 4 * (pg + 1) >= pqc + 1:
                pending_norm[(pqc, ppair)] = emit_normalize(pqc, ppair)
        while strip_ptr[0] < len(strip_micro):
            _emit_next()
        key = (qc_order[-1], 2)
        if key in pending_norm:
            emit_transpose(key[0], key[1], pending_norm.pop(key))
        tt = qc_order[-1]
        ob = ob_pool.tile([128, C], F32, tag="ob")
        po0 = ps_m.tile([128, 512], F32, tag="m")
        for c4 in range(3):
            nc.tensor.matmul(po0[:, :], avT[:, c4, 128 * tt:128 * (tt + 1)],
                             wp[:, c4, 0:512], start=(c4 == 0), stop=False,
                             skip_group_check=True)
        drain_transposes(NQC * NPAIR)
        nc.tensor.matmul(po0[:, :], avT[:, 3, 128 * tt:128 * (tt + 1)],
                         wp[:, 3, 0:512], start=False, stop=True,
                         skip_group_check=True)
        nc.vector.tensor_copy(ob[:, 0:512], po0[:, :])
        nc.sync.dma_start(out_d[128 * tt:128 * (tt + 1), 0:512], ob[:, 0:512])
        po1 = ps_m.tile([128, 512], F32, tag="m")
        for c4 in range(GC // 128):
            nc.tensor.matmul(po1[:, :], avT[:, c4, 128 * tt:128 * (tt + 1)],
                             wp[:, c4, 512:1024], start=(c4 == 0),
                             stop=(c4 == GC // 128 - 1), skip_group_check=True)
        # quarter-granular epilogue so the store pipeline drains sooner
        for qtr in range(2):
            lo, hi = 512 + 256 * qtr, 512 + 256 * (qtr + 1)
            nc.vector.tensor_copy(ob[:, lo:hi], po1[:, lo - 512:hi - 512])
            nc.sync.dma_start(out_d[128 * tt:128 * (tt + 1), lo:hi], ob[:, lo:hi])

    if split_waits:
        _split_excess_waits(nc)
    return nc


def _get_program():
    global _PROGRAM
    if _PROGRAM is None:
        _PROGRAM = _build_program()
    return _PROGRAM


def _make_in_maps(x, Wk, Wq, Wv, Wp):
    import ml_dtypes
    bf = ml_dtypes.bfloat16
    xdt = ml_dtypes.float8_e4m3 if USE_FP8_QKV else bf
    ws = WSCALE if USE_FP8_QKV else 1.0
    x = np.asarray(x, dtype=np.float32)
    Wk = np.asarray(Wk, dtype=np.float32)
    Wq = np.asarray(Wq, dtype=np.float32)
    Wv = np.asarray(Wv, dtype=np.float32)
    Wp = np.asarray(Wp, dtype=np.float32)
    in_maps = []
    for core in range(8):
        b, g = core // GROUPS, core % GROUPS
        rows = slice(GC * g, GC * (g + 1))
        in_maps.append({
            "xT": np.ascontiguousarray(x[b].T).astype(xdt),           # [C, T]
            "wqT": np.ascontiguousarray(Wq[rows, :].T * ws).astype(xdt),
            "wkT": np.ascontiguousarray(Wk[rows, :].T * ws).astype(xdt),
            "wvT": np.ascontiguousarray(Wv[rows, :].T * ws).astype(xdt),
            "wpT": np.ascontiguousarray(Wp[:, rows].T).astype(bf),    # [GC, C]
        })
    return in_maps


def run(x, Wk, Wq, Wv, Wp, bp, trace=False, **spmd_kwargs):
    nc = _get_program()
    in_maps = _make_in_maps(x, Wk, Wq, Wv, Wp)
    res = run_bass_kernel_spmd(nc, in_maps, list(range(8)), trace=trace, **spmd_kwargs)
    bp = np.asarray(bp, dtype=np.float32)
    out = np.empty((B, T, C), dtype=np.float32)
    for b in range(B):
        out[b] = res.results[GROUPS * b]["outp"] + res.results[GROUPS * b + 1]["outp"] + bp
    return out, res


def kernel(x, Wk, Wq, Wv, Wp, bp):
    out, _ = run(x, Wk, Wq, Wv, Wp, bp)
    return out
